# revision 63
# baseline (speedup 1.0000x reference)
"""Multi-head attention (S=2048, B=2, D=1024, H=16) on 8 Trainium2 NeuronCores.

Sharding: batch*head parallel. Core c owns heads [2c, 2c+1]: it holds the
128-column slice of Wq/Wk/Wv and the matching 128-row slice of Wo, computes
its heads' attention over all tokens, and produces a partial output
projection. Partials are summed on the host (the all-reduce step).

Device layout: activations live transposed (features on partitions, tokens
on the free axis) the whole way through:
  - projections:  QT/KT/VT[oc, t] = W_slice.T @ xT        (oc = head-slice col)
  - scores (transposed): sT[j, i]  = KT_j.T @ QT           (j = key pos chunk)
  - softmax: exp on ScalarE over [128, 1024] tiles (both heads at once to
    amortize the ACT fixed overhead); the row-sum over j comes from an extra
    all-ones column appended to V in the AV matmul; normalization by 1/sum is
    a fast-approx reciprocal + GpSimd partition-broadcast + vector multiply.
  - AV:  ctxT[dk, i] (+ sum row) = [V | 1].T @ exp(sT)     (V in natural [j, dk])
  - output: outT[e, t] = Wo_slice.T @ ctxT

The middle ~140us is ScalarE-paced: ACT is a fixed 1 elem/cycle/lane spline
engine ((N+352)/1.2 ns), DVE has no exp, and PSUM capacity (8 x 2KB banks;
TRN2 matmul output must be fp32) caps the exp tile at [128,1024]. So the
whole program is built around keeping the ~1.11us/j-step exp stream
stall-free and hiding everything else in its ~360ns/step tensor slack:
  - Per-engine instruction streams are IN ORDER at runtime and program order
    is the static scheduler's priority. Filler work (other-batch projections,
    output projection) must be either left after the attention loops (the
    scheduler weaves it into idle slots) or woven explicitly 1-2 ops per
    j-step; a contiguous block placed mid-stream stalls the exp pipe, and
    filler that waits on a DMA head-of-line blocks attention.
  - x loads run k,v,q: attention needs K fully and Q's first token group, so
    Q gates the start either way, while V landing second lets V's projection
    + the V-transpose relayout run in the DMA shadow instead of eating
    attention(0)'s slack. Projections are per-token-group chains (8 matmuls
    + bias drain) so Q tg1-3 chains weave as leading filler.
  - eg boundaries: ctx PSUM banks drain to SBUF right after the last AV (two
    vector copies; the sum row goes to a partition-0 tile --
    reciprocal_approx_fast silently corrupts on partition-offset inputs),
    freeing the bank ~2us before the reciprocal/broadcast/multiply chain
    finishes.
Output is staged per 128-row chunk into [128, 2048] SBUF tiles and written
with wide-line DMAs (narrow 1KB-line DMAs measured ~100GB/s); the tail batch
goes tg-outer (its last eg's norm gates only the final 8 matmuls), rotates
PSUM tiles through the then-idle qk ring, alternates ScalarE/DVE casts, and
writes in two waves so the drain overlaps the casts.

All matmuls run in float16 operands (fp32 PSUM accumulation).
"""

import math

import numpy as np

SEQ, BATCH, D_MODEL, HEADS = 2048, 2, 1024, 16
D_K = D_MODEL // HEADS  # 64
N_CORES = 8
HPC = HEADS // N_CORES  # heads per core: 2
OC = HPC * D_K  # per-core head-slice width: 128

LAST_RESULTS = None  # BassKernelResults of the most recent kernel() call


def build_program(S=SEQ, B=BATCH, D=D_MODEL, H=HEADS, n_cores=N_CORES):
    """Build + compile the per-core bass program (SPMD: same program on all
    cores, per-core weight slices arrive via the input maps)."""
    import concourse.bass as bass  # noqa: F401
    import concourse.mybir as mybir
    import concourse.tile as tile
    from concourse import bacc
    from concourse.masks import make_identity

    dk = D // H
    hpc = H // n_cores
    oc = hpc * dk
    T = B * S
    P = 128
    assert oc == P, "kernel assumes a 128-wide per-core head slice"
    assert hpc == 2
    scale = 1.0 / math.sqrt(dk)

    NDC = D // P  # contraction chunks for the projections
    NJC = S // P  # key-position chunks per (batch, head)
    TGW = 512  # token-group width (projections / output)
    NTG_B = S // TGW  # token groups per batch
    EW = 512  # attention i-group width
    NEG = S // EW
    NEC = D // P  # output-projection column chunks

    f32 = mybir.dt.float32
    f16 = mybir.dt.float16
    EXP = mybir.ActivationFunctionType.Exp
    COPY = mybir.ActivationFunctionType.Copy

    nc = bacc.Bacc("TRN2", target_bir_lowering=False, debug=False)

    qT = nc.dram_tensor("qT", (D, T), f16, kind="ExternalInput")
    kT = nc.dram_tensor("kT", (D, T), f16, kind="ExternalInput")
    vT = nc.dram_tensor("vT", (D, T), f16, kind="ExternalInput")
    # weights arrive host-pre-permuted as [p, dc, oc] so one DMA fills the
    # SBUF layout (the sync engine's 620ns per-dma_start issue cost gates
    # the kernel head).
    Wq_s = nc.dram_tensor("Wq_s", (P, NDC, oc), f16, kind="ExternalInput")
    Wk_s = nc.dram_tensor("Wk_s", (P, NDC, oc), f16, kind="ExternalInput")
    Wv_s = nc.dram_tensor("Wv_s", (P, NDC, oc), f16, kind="ExternalInput")
    bq_s = nc.dram_tensor("bq_s", (oc, 1), f32, kind="ExternalInput")
    bk_s = nc.dram_tensor("bk_s", (oc, 1), f32, kind="ExternalInput")
    bv_s = nc.dram_tensor("bv_s", (oc, 1), f32, kind="ExternalInput")
    Wo_s = nc.dram_tensor("Wo_s", (oc, D), f16, kind="ExternalInput")
    outT = nc.dram_tensor("outT", (D, T), f16, kind="ExternalOutput")

    with tile.TileContext(nc) as tc:
        with (
            tc.tile_pool(name="singles", bufs=1) as singles,
            tc.tile_pool(name="xpool", bufs=16) as xpool,
            tc.tile_pool(name="xpoolv", bufs=8) as xpoolv,
            tc.tile_pool(name="expool", bufs=4) as expool,
            tc.tile_pool(name="small", bufs=4) as small,
            tc.tile_pool(name="outp", bufs=8) as outp,
            # PSUM (8 banks): tag "qk" = [128,1024]f32 (2 banks) x 2 bufs,
            # attention-exclusive (the output-projection tail reuses it);
            # tag "ctx" = [65,512]f32 (1 bank) x 2 bufs; tag "fill" =
            # [128,512]f32 (1 bank) x 2 bufs for proj chains / V transposes /
            # outproj so those phases never block attention.
            tc.tile_pool(name="psum", bufs=2, space=bass.MemorySpace.PSUM) as psum,
        ):
            ident = singles.tile([P, P], f16)
            make_identity(nc, ident)

            # (no HAM warm-up: the 4/8 clock gate is harmless during the
            # DMA-paced head -- half-rate projection still beats the DMA
            # pace -- and the dense attention stream re-earns the full-rate
            # grant within ~3 steps; throwaway matmuls only delayed the PE
            # queue behind them)

            wq_sb = singles.tile([P, NDC, oc], f16)
            wk_sb = singles.tile([P, NDC, oc], f16)
            wv_sb = singles.tile([P, NDC, oc], f16)
            wo_sb = singles.tile([oc, D], f16)
            bq_sb = singles.tile([oc, 1], f32)
            bk_sb = singles.tile([oc, 1], f32)
            bv_sb = singles.tile([oc, 1], f32)

            QT_sb = singles.tile([oc, T], f16)
            KT_sb = singles.tile([oc, T], f16)
            VT_sb = singles.tile([oc, T], f16)
            ctxT_sb = singles.tile([oc, T], f16)
            # [V | ones] stationary operands: per (batch, j-chunk) a
            # [128 tokens, hpc*(dk+1)] block, head h at cols h*(dk+1).
            vaug_sb = singles.tile([P, B * NJC, hpc * (dk + 1)], f16)
            ones_cols = singles.tile([P, B * NJC, 1], f32)

            PROJ_TBL = {
                "k": (lambda: (wk_sb, bk_sb, KT_sb)),
                "q": (lambda: (wq_sb, bq_sb, QT_sb)),
                "v": (lambda: (wv_sb, bv_sb, VT_sb)),
            }
            W_DRAM = {"k": (Wk_s, bk_s), "q": (Wq_s, bq_s), "v": (Wv_s, bv_s)}
            W_SB = {"k": (wk_sb, bk_sb), "q": (wq_sb, bq_sb), "v": (wv_sb, bv_sb)}

            def issue_x_loads(b, order, with_weights=False):
                """Issue one batch's input DMAs (8 tiles per tensor; one big
                DMA per tensor would land on too few DMA queues and lose the
                ~420GB/s aggregate). For batch 0 each tensor's weight issues
                just before its x tiles."""
                xts = {}
                for name in order:
                    if with_weights:
                        # weights issue from GPSIMD's DGE path: it fires
                        # immediately (fresh tiles, no slot waits), keeping
                        # 7 x 620ns of issue cost off the sync queue so the
                        # x-tile DMAs start ~4us earlier
                        w_sb, b_sb = W_SB[name]
                        w_dram, b_dram = W_DRAM[name]
                        nc.gpsimd.dma_start(w_sb[:, :, :], w_dram[:, :, :])
                        nc.gpsimd.dma_start(b_sb, b_dram[:, :])
                    x_dram = {"k": kT, "q": qT, "v": vT}[name]
                    pool = xpoolv if (b == 1 and name == "v") else xpool
                    tiles = []
                    for dc in range(NDC):
                        xt = pool.tile([P, S], f16, tag="xt", name=f"xt_{name}{b}_{dc}")
                        nc.sync.dma_start(
                            xt, x_dram[dc * P : (dc + 1) * P, b * S : (b + 1) * S]
                        )
                        tiles.append(xt)
                    xts[name] = tiles
                if with_weights:
                    nc.gpsimd.dma_start(wo_sb, Wo_s[:, :])
                return xts

            def proj_tg_ops(b, name, tg, xts):
                """Thunks for one token group of one projection: an 8-matmul
                dc chain into a single fill-ring PSUM tile, then a bias-add
                drain. One instruction per thunk so the chain can weave into
                the attention j-loop; the PSUM tile allocates lazily at emit
                time so ring slots are claimed in true program order."""
                tiles = xts[name]
                box = {}
                for dc in range(NDC):
                    def mm(dc=dc, box=box, name=name, tg=tg):
                        w_sb = PROJ_TBL[name]()[0]
                        if "ps" not in box:
                            box["ps"] = psum.tile(
                                [oc, TGW], f32, tag="fill", name=f"ps_{name}{b}_{tg}"
                            )
                        nc.tensor.matmul(
                            box["ps"],
                            w_sb[:, dc, :],
                            tiles[dc][:, tg * TGW : (tg + 1) * TGW],
                            start=(dc == 0),
                            stop=(dc == NDC - 1),
                        )
                    yield mm
                def drain(box=box, name=name, tg=tg):
                    b_sb, dstT = PROJ_TBL[name]()[1:]
                    t0 = b * S + tg * TGW
                    nc.vector.tensor_scalar_add(dstT[:, t0 : t0 + TGW], box["ps"], b_sb[:])
                yield drain

            def run_proj_one(b, name, xts, tgs=None):
                for tg in tgs if tgs is not None else range(NTG_B):
                    for op in proj_tg_ops(b, name, tg, xts):
                        op()

            def vaug_ops(b):
                """Thunks for V's natural layout (+ ones cols): PE transpose
                then two strided head copies per j-chunk."""
                for j in range(NJC):
                    pst_box = {}
                    def tr(j=j, pst_box=pst_box):
                        pst_box[0] = psum.tile([P, P], f16, tag="fill", name=f"pst{b}_{j}")
                        nc.tensor.transpose(
                            pst_box[0],
                            VT_sb[:, b * S + j * P : b * S + (j + 1) * P],
                            ident[:],
                        )
                    yield tr
                    for h in range(hpc):
                        def cp(j=j, h=h, pst_box=pst_box):
                            nc.vector.tensor_copy(
                                vaug_sb[:, b * NJC + j, h * (dk + 1) : h * (dk + 1) + dk],
                                pst_box[0][:, h * dk : (h + 1) * dk],
                            )
                        yield cp

            def run_attention(b, egs, filler=None, skip_steps=0):
                """Attention for both heads of batch b. Software-pipelined:
                QK(j+1) issues before AV(j) so exp(j) (ScalarE) is complete by
                the time the tensor engine reaches AV(j). The two heads' K=64
                QK matmuls land in disjoint PE row groups (h0 rows 0-63, h1
                rows 64-127) writing halves of one 1024-wide PSUM tile, and
                run concurrently; one 1024-wide exp covers both heads.
                `filler` is a list of per-step op batches; one batch is
                emitted after each j-step (the slack fits ~1.7 matmuls per
                step -- batches bigger than that stretch the step)."""
                filler = list(filler) if filler else []
                fi = 0
                step = 0
                QTp = [QT_sb[h * dk : (h + 1) * dk, b * S : (b + 1) * S] for h in range(hpc)]
                KTp = [KT_sb[h * dk : (h + 1) * dk, b * S : (b + 1) * S] for h in range(hpc)]
                for eg in egs:
                    i0 = eg * EW
                    ctxs = [
                        psum.tile(
                            [dk + 1, EW], f32, tag="ctx", bufs=2, name=f"ctx{b}_{eg}_{h}"
                        )
                        for h in range(hpc)
                    ]

                    def issue_qk(j):
                        qk2 = psum.tile([P, 2 * EW], f32, tag="qk", name=f"qk{b}_{eg}_{j}")
                        for h in range(hpc):
                            nc.tensor.matmul(
                                qk2[:, h * EW : (h + 1) * EW],
                                KTp[h][:, j * P : (j + 1) * P],
                                QTp[h][:, i0 : i0 + EW],
                                start=True,
                                stop=True,
                            )
                        return qk2

                    def issue_exp_av(j, qk2):
                        ex2 = expool.tile([P, 2 * EW], f16, tag="ex", name="ex2")
                        nc.scalar.activation(ex2[:], qk2[:], EXP, scale=scale)
                        for h in range(hpc):
                            nc.tensor.matmul(
                                ctxs[h],
                                vaug_sb[:, b * NJC + j, h * (dk + 1) : (h + 1) * (dk + 1)],
                                ex2[:, h * EW : (h + 1) * EW],
                                start=(j == 0),
                                stop=(j == NJC - 1),
                            )

                    prev = issue_qk(0)
                    for j in range(1, NJC):
                        cur = issue_qk(j)
                        issue_exp_av(j - 1, prev)
                        prev = cur
                        step += 1
                        if step > skip_steps and fi < len(filler):
                            for op in filler[fi]:
                                op()
                            fi += 1
                    issue_exp_av(NJC - 1, prev)

                    # Drain both ctx PSUM banks to SBUF immediately (frees
                    # them for the next eg's AV chain ~2us sooner than
                    # normalizing from PSUM), then normalize from SBUF.
                    csb = []
                    for h in range(hpc):
                        c = small.tile([dk, EW], f32, tag="ctxsb", bufs=4,
                                       name=f"csb{b}_{eg}_{h}")
                        sr = small.tile([1, EW], f32, tag="srow", bufs=2,
                                        name=f"sr{b}_{eg}_{h}")
                        nc.vector.tensor_copy(c[:], ctxs[h][0:dk, :])
                        nc.vector.tensor_copy(sr[:], ctxs[h][dk : dk + 1, :])
                        csb.append((c, sr))
                    for h in range(hpc):
                        c, sr = csb[h]
                        rec = small.tile([1, EW], f32, tag="rec", bufs=2)
                        nc.vector.reciprocal_approx_fast(rec[:], sr[:])
                        bc = small.tile([dk, EW], f32, tag="bc", bufs=2)
                        nc.gpsimd.partition_broadcast(bc[:], rec[:])
                        nc.vector.tensor_mul(
                            ctxT_sb[h * dk : (h + 1) * dk, b * S + i0 : b * S + i0 + EW],
                            c[:],
                            bc[:],
                        )
                # leftover filler runs here (a block: only safe at the end
                # of a batch where the next phase tolerates a short stall)
                while fi < len(filler):
                    for op in filler[fi]:
                        op()
                    fi += 1

            def outproj_fill_ops(b):
                """Thunks for the ready batch's output projection, woven into
                the other batch's attention. ec-outer: each 128-row chunk
                accumulates 4 token-group casts into one [128, 2048] SBUF
                tile, then one DMA (4KB lines). Casts on DVE (ScalarE is the
                exp bottleneck)."""
                for ec in range(NEC):
                    box = {}
                    for tg in range(NTG_B):
                        def mm(ec=ec, tg=tg, box=box):
                            if "ot" not in box:
                                box["ot"] = outp.tile([P, S], f16, tag="ot", name=f"ot{b}_{ec}")
                            t0 = b * S + tg * TGW
                            box[tg] = psum.tile(
                                [P, TGW], f32, tag="fill", name=f"ps2_{b}_{ec}_{tg}"
                            )
                            nc.tensor.matmul(
                                box[tg],
                                wo_sb[:, ec * P : (ec + 1) * P],
                                ctxT_sb[:, t0 : t0 + TGW],
                                start=True,
                                stop=True,
                            )
                        yield mm
                        def cast(ec=ec, tg=tg, box=box):
                            nc.vector.tensor_copy(
                                box["ot"][:, tg * TGW : (tg + 1) * TGW], box[tg][:]
                            )
                        yield cast
                    def wr(ec=ec, box=box):
                        nc.sync.dma_start(
                            outT[ec * P : (ec + 1) * P, b * S : (b + 1) * S], box["ot"][:]
                        )
                    yield wr

            def outproj_tail_tg0_ops(b, ots):
                """Thunks for the tail batch's first token group (8 matmuls +
                DVE casts), woven into the tail end of the other-batch
                attention: its ctxT is ready after eg0 and the fill ring is
                free once the fill-batch output projection has drained, so
                only token groups 1-3 remain after the last exp."""
                for ec in range(NEC):
                    box = {}
                    def mm(ec=ec, box=box):
                        # ots allocate here, AFTER the fill batch's 8 "ot"
                        # allocations: these 8 live until the final waves, so
                        # allocating them first would deadlock the 8-slot ring
                        if not ots:
                            for e in range(NEC):
                                ots.append(
                                    outp.tile([P, S], f16, tag="ot", name=f"ot{b}_{e}")
                                )
                        box[0] = psum.tile(
                            [P, TGW], f32, tag="fill", name=f"ps2_{b}_{ec}_0"
                        )
                        nc.tensor.matmul(
                            box[0],
                            wo_sb[:, ec * P : (ec + 1) * P],
                            ctxT_sb[:, b * S : b * S + TGW],
                            start=True,
                            stop=True,
                        )
                    yield mm
                    def cast(ec=ec, box=box):
                        nc.vector.tensor_copy(ots[ec][:, 0:TGW], box[0][:])
                    yield cast

            def run_outproj_tail(b, ots):
                """Output projection tail, tg-outer so only the final 8
                matmuls wait on the last eg's normalization. PSUM rotates
                through the idle qk+fill rings (4 in flight: no matmul<->cast
                ping-pong, PE stays busy enough to hold full HAM rate); casts
                alternate ScalarE/DVE; writes go in two [128,1024] waves
                (2KB lines) so the drain overlaps the remaining casts."""
                idx = 0
                for tg in range(1, NTG_B):
                    t0 = b * S + tg * TGW
                    for ec in range(NEC):
                        ps2 = psum.tile(
                            [P, TGW], f32,
                            tag=("qk" if idx % 2 == 0 else "fill"),
                            name=f"ps2_{b}_{ec}_{tg}",
                        )
                        idx += 1
                        nc.tensor.matmul(
                            ps2,
                            wo_sb[:, ec * P : (ec + 1) * P],
                            ctxT_sb[:, t0 : t0 + TGW],
                            start=True,
                            stop=True,
                        )
                        dst = ots[ec][:, tg * TGW : (tg + 1) * TGW]
                        if idx % 2 == 0:
                            nc.scalar.activation(dst, ps2[:], COPY)
                        else:
                            nc.vector.tensor_copy(dst, ps2[:])
                    if tg == 1:
                        for ec in range(NEC):
                            nc.sync.dma_start(
                                outT[ec * P : (ec + 1) * P, b * S : b * S + 2 * TGW],
                                ots[ec][:, 0 : 2 * TGW],
                            )
                for ec in range(NEC):
                    nc.sync.dma_start(
                        outT[ec * P : (ec + 1) * P, b * S + 2 * TGW : b * S + 4 * TGW],
                        ots[ec][:, 2 * TGW : 4 * TGW],
                    )

            def vaug_step_filler(b, extra, extra_rate=1):
                """Per-step filler batches for one batch's attention: the
                V-relayout chunks 2..15 (3 ops each, completing 2 steps ahead
                of the AV that reads each chunk), interleaved with `extra`
                ops (one per step), then the remaining extras 2 per step.
                Returns (preblock_ops, step_batches)."""
                vops = list(vaug_ops(b))
                pre, vops = vops[:6], vops[6:]
                extra = list(extra)
                steps = []
                for c in range(14):
                    batch = vops[c * 3 : (c + 1) * 3]
                    for _ in range(min(extra_rate, len(extra))):
                        batch.append(extra.pop(0))
                    steps.append(batch)
                while extra:
                    steps.append([extra.pop(0) for _ in range(min(2, len(extra)))])
                return pre, steps

            # ---- head: k,v,q loads; K/V projections hide in the DMA
            # ---- shadow; Q's first token group gates attention(0).
            xts0 = issue_x_loads(0, ("k", "v", "q"), with_weights=True)
            nc.vector.memset(ones_cols, 1.0)
            # preload the exp activation table set (~2.7us) during the
            # DMA-bound head instead of at the first real exp.
            exp_dummy = singles.tile([P, 1], f16)
            nc.scalar.activation(exp_dummy[:], ones_cols[:, 0, :], EXP)
            for h in range(hpc):
                one_col = h * (dk + 1) + dk
                nc.vector.tensor_copy(vaug_sb[:, :, one_col : one_col + 1], ones_cols[:])

            run_proj_one(0, "k", xts0)
            run_proj_one(0, "v", xts0)
            run_proj_one(0, "q", xts0)
            # attention(0) filler: V-relayout chunks (2 preblocked, rest 3
            # ops/step just ahead of their AVs), Q tg1-3 chains (each done
            # ahead of the eg that reads it), and the x1 DMA batch at eg0's
            # end. proj(1) stays AFTER attention(0) in program order: the
            # static scheduler weaves its chains into remaining slack as x1
            # tiles land.
            pre0, steps0 = vaug_step_filler(0, [], extra_rate=0)
            # x1's tile allocations reuse x0's 16 slots, so each x1 DMA
            # waits until the x0 tiles it replaces have been fully read by
            # every ALREADY-EMITTED reader: x1 never steals HBM bandwidth
            # from x0-q, whose landing gates the first exp. Only x1-k (whose
            # slots' readers, the x0-v chains, are all emitted) may issue
            # before attention(0); x1-v must follow the woven q0 chains and
            # x1-q must follow the k1 projection, else the DMA lands before
            # readers that are emitted later in program order (silent
            # use-after-free -- the dependency tracker only orders against
            # prior program points).
            xts1 = issue_x_loads(1, ("k", "q"))
            xts1.update(issue_x_loads(1, ("v",)))
            for op in pre0:
                op()
            run_attention(0, egs=range(NEG), filler=steps0)
            run_proj_one(1, "k", xts1)
            run_proj_one(1, "q", xts1)
            run_proj_one(1, "v", xts1)
            # k1/v1 projections and q1's first token group are emitted
            # here (the scheduler weaves them into attention(0)'s remaining
            # slack as the x1 tiles land); q1 tg1-3 weave into attention(1)
            # itself, each finishing ahead of the eg that reads it.
            ots1 = []
            ext1 = list(outproj_fill_ops(0))
            ext1.extend(outproj_tail_tg0_ops(1, ots1))
            # attention(1) filler: batch 1's V-relayout first (its AVs need
            # it chunk by chunk), then batch 0's output projection and the
            # tail batch's first token group.
            pre1, steps1 = vaug_step_filler(1, ext1, extra_rate=2)
            for op in pre1:
                op()
            run_attention(1, egs=range(NEG), filler=steps1)
            run_outproj_tail(1, ots1)

    nc.compile()
    return nc


_NC_CACHE = {}


def _compiled():
    if "nc" not in _NC_CACHE:
        _NC_CACHE["nc"] = build_program()
    return _NC_CACHE["nc"]


def _permute_w(w):
    """[D, oc] -> [p, dc, oc] so the device can fetch it with one DMA."""
    D, oc = w.shape
    return np.ascontiguousarray(w.reshape(D // 128, 128, oc).transpose(1, 0, 2))


def make_in_maps(q, k, v, Wq, bq, Wk, bk, Wv, bv, Wo):
    """Shard inputs for the 8 cores: shared transposed activations plus
    per-core head-slice weight columns / Wo rows."""
    f = np.float32
    h16 = np.float16
    qT = np.ascontiguousarray(q.transpose(2, 1, 0).reshape(D_MODEL, -1)).astype(h16)
    kT = np.ascontiguousarray(k.transpose(2, 1, 0).reshape(D_MODEL, -1)).astype(h16)
    vT = np.ascontiguousarray(v.transpose(2, 1, 0).reshape(D_MODEL, -1)).astype(h16)
    Wq, Wk, Wv, Wo = (np.asarray(w).astype(h16) for w in (Wq, Wk, Wv, Wo))
    in_maps = []
    for c in range(N_CORES):
        sl = slice(c * OC, (c + 1) * OC)
        in_maps.append(
            {
                "qT": qT,
                "kT": kT,
                "vT": vT,
                "Wq_s": _permute_w(Wq[:, sl]),
                "Wk_s": _permute_w(Wk[:, sl]),
                "Wv_s": _permute_w(Wv[:, sl]),
                "bq_s": np.ascontiguousarray(bq[sl].reshape(OC, 1), dtype=f),
                "bk_s": np.ascontiguousarray(bk[sl].reshape(OC, 1), dtype=f),
                "bv_s": np.ascontiguousarray(bv[sl].reshape(OC, 1), dtype=f),
                "Wo_s": np.ascontiguousarray(Wo[sl, :]),
            }
        )
    return in_maps


def _install_ntff_shim():
    """Provide antenv.axon_hooks (absent on some images) so that
    trace=True / BASS_TRACE=1 in run_bass_kernel_spmd works instead of
    crashing with ModuleNotFoundError. Best-effort."""
    import sys
    import types

    try:
        import antenv
    except ImportError:
        return
    try:
        import antenv.axon_hooks  # noqa: F401

        return  # real module exists
    except ImportError:
        pass
    mod = types.ModuleType("antenv.axon_hooks")
    mod._hook = None
    mod.set_axon_ntff_profile_hook = lambda h: setattr(mod, "_hook", h)
    mod.get_axon_ntff_profile_hook = lambda: mod._hook
    sys.modules["antenv.axon_hooks"] = mod
    antenv.axon_hooks = mod
    try:
        from trn_agent_boot.trn_boot import _ntff_profile_via_ctypes

        hook = _ntff_profile_via_ctypes("/opt/axon/libaxon_pjrt.so")
        if hook is not None:
            mod.set_axon_ntff_profile_hook(hook)
    except Exception:  # noqa: BLE001
        pass


def kernel(q, k, v, Wq, bq, Wk, bk, Wv, bv, Wo, bo):
    global LAST_RESULTS
    from concourse.bass_utils import run_bass_kernel_spmd

    _install_ntff_shim()

    nc = _compiled()
    in_maps = make_in_maps(q, k, v, Wq, bq, Wk, bk, Wv, bv, Wo)
    res = run_bass_kernel_spmd(nc, in_maps, core_ids=list(range(N_CORES)))
    LAST_RESULTS = res
    total = res.results[0]["outT"].astype(np.float64)
    for c in range(1, N_CORES):
        total += res.results[c]["outT"]
    out = total.reshape(D_MODEL, BATCH, SEQ).transpose(2, 1, 0) + np.asarray(
        bo, dtype=np.float64
    )
    return np.ascontiguousarray(out, dtype=np.float32)


# revision 64
# speedup vs baseline: 1.0034x; 1.0034x over previous
"""Multi-head attention (S=2048, B=2, D=1024, H=16) on 8 Trainium2 NeuronCores.

Sharding: batch*head parallel. Core c owns heads [2c, 2c+1]: it holds the
128-column slice of Wq/Wk/Wv and the matching 128-row slice of Wo, computes
its heads' attention over all tokens, and produces a partial output
projection. Partials are summed on the host (the all-reduce step).

Device layout: activations live transposed (features on partitions, tokens
on the free axis) the whole way through:
  - projections:  QT/KT/VT[oc, t] = W_slice.T @ xT        (oc = head-slice col)
  - scores (transposed): sT[j, i]  = KT_j.T @ QT           (j = key pos chunk)
  - softmax: exp on ScalarE over [128, 1024] tiles (both heads at once to
    amortize the ACT fixed overhead); the row-sum over j comes from an extra
    all-ones column appended to V in the AV matmul; normalization by 1/sum is
    a fast-approx reciprocal + GpSimd partition-broadcast + vector multiply.
  - AV:  ctxT[dk, i] (+ sum row) = [V | 1].T @ exp(sT)     (V in natural [j, dk])
  - output: outT[e, t] = Wo_slice.T @ ctxT

The middle ~140us is ScalarE-paced: ACT is a fixed 1 elem/cycle/lane spline
engine ((N+352)/1.2 ns), DVE has no exp, and PSUM capacity (8 x 2KB banks;
TRN2 matmul output must be fp32) caps the exp tile at [128,1024]. So the
whole program is built around keeping the ~1.11us/j-step exp stream
stall-free and hiding everything else in its ~360ns/step tensor slack:
  - Per-engine instruction streams are IN ORDER at runtime and program order
    is the static scheduler's priority. Filler work (other-batch projections,
    output projection) must be either left after the attention loops (the
    scheduler weaves it into idle slots) or woven explicitly 1-2 ops per
    j-step; a contiguous block placed mid-stream stalls the exp pipe, and
    filler that waits on a DMA head-of-line blocks attention.
  - x loads run k,v,q: attention needs K fully and Q's first token group, so
    Q gates the start either way, while V landing second lets V's projection
    + the V-transpose relayout run in the DMA shadow instead of eating
    attention(0)'s slack. Projections are per-token-group chains (8 matmuls
    + bias drain) so Q tg1-3 chains weave as leading filler.
  - eg boundaries: ctx PSUM banks drain to SBUF right after the last AV (two
    vector copies; the sum row goes to a partition-0 tile --
    reciprocal_approx_fast silently corrupts on partition-offset inputs),
    freeing the bank ~2us before the reciprocal/broadcast/multiply chain
    finishes.
Output is staged per 128-row chunk into [128, 2048] SBUF tiles and written
with wide-line DMAs (narrow 1KB-line DMAs measured ~100GB/s); the tail batch
goes tg-outer (its last eg's norm gates only the final 8 matmuls), rotates
PSUM tiles through the then-idle qk ring, alternates ScalarE/DVE casts, and
writes in two waves so the drain overlaps the casts.

All matmuls run in float16 operands (fp32 PSUM accumulation).
"""

import math

import numpy as np

SEQ, BATCH, D_MODEL, HEADS = 2048, 2, 1024, 16
D_K = D_MODEL // HEADS  # 64
N_CORES = 8
HPC = HEADS // N_CORES  # heads per core: 2
OC = HPC * D_K  # per-core head-slice width: 128

LAST_RESULTS = None  # BassKernelResults of the most recent kernel() call


def build_program(S=SEQ, B=BATCH, D=D_MODEL, H=HEADS, n_cores=N_CORES):
    """Build + compile the per-core bass program (SPMD: same program on all
    cores, per-core weight slices arrive via the input maps)."""
    import concourse.bass as bass  # noqa: F401
    import concourse.mybir as mybir
    import concourse.tile as tile
    from concourse import bacc
    from concourse.masks import make_identity

    dk = D // H
    hpc = H // n_cores
    oc = hpc * dk
    T = B * S
    P = 128
    assert oc == P, "kernel assumes a 128-wide per-core head slice"
    assert hpc == 2
    scale = 1.0 / math.sqrt(dk)

    NDC = D // P  # contraction chunks for the projections
    NJC = S // P  # key-position chunks per (batch, head)
    TGW = 512  # token-group width (projections / output)
    NTG_B = S // TGW  # token groups per batch
    EW = 512  # attention i-group width
    NEG = S // EW
    NEC = D // P  # output-projection column chunks

    f32 = mybir.dt.float32
    f16 = mybir.dt.float16
    EXP = mybir.ActivationFunctionType.Exp
    COPY = mybir.ActivationFunctionType.Copy

    nc = bacc.Bacc("TRN2", target_bir_lowering=False, debug=False)

    qT = nc.dram_tensor("qT", (D, T), f16, kind="ExternalInput")
    kT = nc.dram_tensor("kT", (D, T), f16, kind="ExternalInput")
    vT = nc.dram_tensor("vT", (D, T), f16, kind="ExternalInput")
    # weights arrive host-pre-permuted as [p, dc, oc] so one DMA fills the
    # SBUF layout (the sync engine's 620ns per-dma_start issue cost gates
    # the kernel head).
    Wq_s = nc.dram_tensor("Wq_s", (P, NDC, oc), f16, kind="ExternalInput")
    Wk_s = nc.dram_tensor("Wk_s", (P, NDC, oc), f16, kind="ExternalInput")
    Wv_s = nc.dram_tensor("Wv_s", (P, NDC, oc), f16, kind="ExternalInput")
    bq_s = nc.dram_tensor("bq_s", (oc, 1), f32, kind="ExternalInput")
    bk_s = nc.dram_tensor("bk_s", (oc, 1), f32, kind="ExternalInput")
    bv_s = nc.dram_tensor("bv_s", (oc, 1), f32, kind="ExternalInput")
    Wo_s = nc.dram_tensor("Wo_s", (oc, D), f16, kind="ExternalInput")
    outT = nc.dram_tensor("outT", (D, T), f16, kind="ExternalOutput")

    with tile.TileContext(nc) as tc:
        with (
            tc.tile_pool(name="singles", bufs=1) as singles,
            tc.tile_pool(name="xpool", bufs=16) as xpool,
            tc.tile_pool(name="xpoolv", bufs=8) as xpoolv,
            tc.tile_pool(name="expool", bufs=4) as expool,
            tc.tile_pool(name="small", bufs=4) as small,
            tc.tile_pool(name="outp", bufs=8) as outp,
            # PSUM (8 banks): tag "qk" = [128,1024]f32 (2 banks) x 2 bufs,
            # attention-exclusive (the output-projection tail reuses it);
            # tag "ctx" = [65,512]f32 (1 bank) x 2 bufs; tag "fill" =
            # [128,512]f32 (1 bank) x 2 bufs for proj chains / V transposes /
            # outproj so those phases never block attention.
            tc.tile_pool(name="psum", bufs=2, space=bass.MemorySpace.PSUM) as psum,
        ):
            ident = singles.tile([P, P], f16)
            make_identity(nc, ident)

            # HAM warm-up: back-to-back throwaway matmuls seed the PE's
            # activity monitor (the 4/8 clock gate wants ~3.4us of sustained
            # work) before the DMA-paced projection phase begins.
            warm_ps = psum.tile([P, 2 * EW], f32, tag="qk", name="warm_ps")
            for i in range(10):
                r = (i % 8) * P
                nc.tensor.matmul(
                    warm_ps[:, r : r + P], ident[:], ident[:], start=True, stop=True
                )

            wq_sb = singles.tile([P, NDC, oc], f16)
            wk_sb = singles.tile([P, NDC, oc], f16)
            wv_sb = singles.tile([P, NDC, oc], f16)
            wo_sb = singles.tile([oc, D], f16)
            bq_sb = singles.tile([oc, 1], f32)
            bk_sb = singles.tile([oc, 1], f32)
            bv_sb = singles.tile([oc, 1], f32)

            QT_sb = singles.tile([oc, T], f16)
            KT_sb = singles.tile([oc, T], f16)
            VT_sb = singles.tile([oc, T], f16)
            ctxT_sb = singles.tile([oc, T], f16)
            # [V | ones] stationary operands: per (batch, j-chunk) a
            # [128 tokens, hpc*(dk+1)] block, head h at cols h*(dk+1).
            vaug_sb = singles.tile([P, B * NJC, hpc * (dk + 1)], f16)
            ones_cols = singles.tile([P, B * NJC, 1], f32)

            PROJ_TBL = {
                "k": (lambda: (wk_sb, bk_sb, KT_sb)),
                "q": (lambda: (wq_sb, bq_sb, QT_sb)),
                "v": (lambda: (wv_sb, bv_sb, VT_sb)),
            }
            W_DRAM = {"k": (Wk_s, bk_s), "q": (Wq_s, bq_s), "v": (Wv_s, bv_s)}
            W_SB = {"k": (wk_sb, bk_sb), "q": (wq_sb, bq_sb), "v": (wv_sb, bv_sb)}

            def issue_x_loads(b, order, with_weights=False):
                """Issue one batch's input DMAs (8 tiles per tensor; one big
                DMA per tensor would land on too few DMA queues and lose the
                ~420GB/s aggregate). For batch 0 each tensor's weight issues
                just before its x tiles."""
                xts = {}
                for name in order:
                    if with_weights:
                        # weights issue from GPSIMD's DGE path: it fires
                        # immediately (fresh tiles, no slot waits), keeping
                        # 7 x 620ns of issue cost off the sync queue so the
                        # x-tile DMAs start ~4us earlier
                        w_sb, b_sb = W_SB[name]
                        w_dram, b_dram = W_DRAM[name]
                        nc.gpsimd.dma_start(w_sb[:, :, :], w_dram[:, :, :])
                        nc.gpsimd.dma_start(b_sb, b_dram[:, :])
                    x_dram = {"k": kT, "q": qT, "v": vT}[name]
                    pool = xpoolv if (b == 1 and name == "v") else xpool
                    tiles = []
                    for dc in range(NDC):
                        xt = pool.tile([P, S], f16, tag="xt", name=f"xt_{name}{b}_{dc}")
                        nc.sync.dma_start(
                            xt, x_dram[dc * P : (dc + 1) * P, b * S : (b + 1) * S]
                        )
                        tiles.append(xt)
                    xts[name] = tiles
                if with_weights:
                    nc.gpsimd.dma_start(wo_sb, Wo_s[:, :])
                return xts

            def proj_tg_ops(b, name, tg, xts):
                """Thunks for one token group of one projection: an 8-matmul
                dc chain into a single fill-ring PSUM tile, then a bias-add
                drain. One instruction per thunk so the chain can weave into
                the attention j-loop; the PSUM tile allocates lazily at emit
                time so ring slots are claimed in true program order."""
                tiles = xts[name]
                box = {}
                for dc in range(NDC):
                    def mm(dc=dc, box=box, name=name, tg=tg):
                        w_sb = PROJ_TBL[name]()[0]
                        if "ps" not in box:
                            box["ps"] = psum.tile(
                                [oc, TGW], f32, tag="fill", name=f"ps_{name}{b}_{tg}"
                            )
                        nc.tensor.matmul(
                            box["ps"],
                            w_sb[:, dc, :],
                            tiles[dc][:, tg * TGW : (tg + 1) * TGW],
                            start=(dc == 0),
                            stop=(dc == NDC - 1),
                        )
                    yield mm
                def drain(box=box, name=name, tg=tg):
                    b_sb, dstT = PROJ_TBL[name]()[1:]
                    t0 = b * S + tg * TGW
                    nc.vector.tensor_scalar_add(dstT[:, t0 : t0 + TGW], box["ps"], b_sb[:])
                yield drain

            def run_proj_one(b, name, xts, tgs=None):
                for tg in tgs if tgs is not None else range(NTG_B):
                    for op in proj_tg_ops(b, name, tg, xts):
                        op()

            def vaug_ops(b):
                """Thunks for V's natural layout (+ ones cols): PE transpose
                then two strided head copies per j-chunk."""
                for j in range(NJC):
                    pst_box = {}
                    def tr(j=j, pst_box=pst_box):
                        pst_box[0] = psum.tile([P, P], f16, tag="fill", name=f"pst{b}_{j}")
                        nc.tensor.transpose(
                            pst_box[0],
                            VT_sb[:, b * S + j * P : b * S + (j + 1) * P],
                            ident[:],
                        )
                    yield tr
                    for h in range(hpc):
                        def cp(j=j, h=h, pst_box=pst_box):
                            nc.vector.tensor_copy(
                                vaug_sb[:, b * NJC + j, h * (dk + 1) : h * (dk + 1) + dk],
                                pst_box[0][:, h * dk : (h + 1) * dk],
                            )
                        yield cp

            def run_attention(b, egs, filler=None, skip_steps=0):
                """Attention for both heads of batch b. Software-pipelined:
                QK(j+1) issues before AV(j) so exp(j) (ScalarE) is complete by
                the time the tensor engine reaches AV(j). The two heads' K=64
                QK matmuls land in disjoint PE row groups (h0 rows 0-63, h1
                rows 64-127) writing halves of one 1024-wide PSUM tile, and
                run concurrently; one 1024-wide exp covers both heads.
                `filler` is a list of per-step op batches; one batch is
                emitted after each j-step (the slack fits ~1.7 matmuls per
                step -- batches bigger than that stretch the step)."""
                filler = list(filler) if filler else []
                fi = 0
                step = 0
                QTp = [QT_sb[h * dk : (h + 1) * dk, b * S : (b + 1) * S] for h in range(hpc)]
                KTp = [KT_sb[h * dk : (h + 1) * dk, b * S : (b + 1) * S] for h in range(hpc)]
                for eg in egs:
                    i0 = eg * EW
                    ctxs = [
                        psum.tile(
                            [dk + 1, EW], f32, tag="ctx", bufs=2, name=f"ctx{b}_{eg}_{h}"
                        )
                        for h in range(hpc)
                    ]

                    def issue_qk(j):
                        qk2 = psum.tile([P, 2 * EW], f32, tag="qk", name=f"qk{b}_{eg}_{j}")
                        for h in range(hpc):
                            nc.tensor.matmul(
                                qk2[:, h * EW : (h + 1) * EW],
                                KTp[h][:, j * P : (j + 1) * P],
                                QTp[h][:, i0 : i0 + EW],
                                start=True,
                                stop=True,
                            )
                        return qk2

                    def issue_exp_av(j, qk2):
                        ex2 = expool.tile([P, 2 * EW], f16, tag="ex", name="ex2")
                        nc.scalar.activation(ex2[:], qk2[:], EXP, scale=scale)
                        for h in range(hpc):
                            nc.tensor.matmul(
                                ctxs[h],
                                vaug_sb[:, b * NJC + j, h * (dk + 1) : (h + 1) * (dk + 1)],
                                ex2[:, h * EW : (h + 1) * EW],
                                start=(j == 0),
                                stop=(j == NJC - 1),
                            )

                    prev = issue_qk(0)
                    for j in range(1, NJC):
                        cur = issue_qk(j)
                        issue_exp_av(j - 1, prev)
                        prev = cur
                        step += 1
                        if step > skip_steps and fi < len(filler):
                            for op in filler[fi]:
                                op()
                            fi += 1
                    issue_exp_av(NJC - 1, prev)

                    # Drain both ctx PSUM banks to SBUF immediately (frees
                    # them for the next eg's AV chain ~2us sooner than
                    # normalizing from PSUM), then normalize from SBUF.
                    csb = []
                    for h in range(hpc):
                        c = small.tile([dk, EW], f32, tag="ctxsb", bufs=4,
                                       name=f"csb{b}_{eg}_{h}")
                        sr = small.tile([1, EW], f32, tag="srow", bufs=2,
                                        name=f"sr{b}_{eg}_{h}")
                        nc.vector.tensor_copy(c[:], ctxs[h][0:dk, :])
                        nc.vector.tensor_copy(sr[:], ctxs[h][dk : dk + 1, :])
                        csb.append((c, sr))
                    for h in range(hpc):
                        c, sr = csb[h]
                        rec = small.tile([1, EW], f32, tag="rec", bufs=2)
                        nc.vector.reciprocal_approx_fast(rec[:], sr[:])
                        bc = small.tile([dk, EW], f32, tag="bc", bufs=2)
                        nc.gpsimd.partition_broadcast(bc[:], rec[:])
                        nc.vector.tensor_mul(
                            ctxT_sb[h * dk : (h + 1) * dk, b * S + i0 : b * S + i0 + EW],
                            c[:],
                            bc[:],
                        )
                # leftover filler runs here (a block: only safe at the end
                # of a batch where the next phase tolerates a short stall)
                while fi < len(filler):
                    for op in filler[fi]:
                        op()
                    fi += 1

            def outproj_fill_ops(b):
                """Thunks for the ready batch's output projection, woven into
                the other batch's attention. ec-outer: each 128-row chunk
                accumulates 4 token-group casts into one [128, 2048] SBUF
                tile, then one DMA (4KB lines). Casts on DVE (ScalarE is the
                exp bottleneck)."""
                for ec in range(NEC):
                    box = {}
                    for tg in range(NTG_B):
                        def mm(ec=ec, tg=tg, box=box):
                            if "ot" not in box:
                                box["ot"] = outp.tile([P, S], f16, tag="ot", name=f"ot{b}_{ec}")
                            t0 = b * S + tg * TGW
                            box[tg] = psum.tile(
                                [P, TGW], f32, tag="fill", name=f"ps2_{b}_{ec}_{tg}"
                            )
                            nc.tensor.matmul(
                                box[tg],
                                wo_sb[:, ec * P : (ec + 1) * P],
                                ctxT_sb[:, t0 : t0 + TGW],
                                start=True,
                                stop=True,
                            )
                        yield mm
                        def cast(ec=ec, tg=tg, box=box):
                            nc.vector.tensor_copy(
                                box["ot"][:, tg * TGW : (tg + 1) * TGW], box[tg][:]
                            )
                        yield cast
                    def wr(ec=ec, box=box):
                        nc.sync.dma_start(
                            outT[ec * P : (ec + 1) * P, b * S : (b + 1) * S], box["ot"][:]
                        )
                    yield wr

            def outproj_tail_tg0_ops(b, ots):
                """Thunks for the tail batch's first token group (8 matmuls +
                DVE casts), woven into the tail end of the other-batch
                attention: its ctxT is ready after eg0 and the fill ring is
                free once the fill-batch output projection has drained, so
                only token groups 1-3 remain after the last exp."""
                for ec in range(NEC):
                    box = {}
                    def mm(ec=ec, box=box):
                        # ots allocate here, AFTER the fill batch's 8 "ot"
                        # allocations: these 8 live until the final waves, so
                        # allocating them first would deadlock the 8-slot ring
                        if not ots:
                            for e in range(NEC):
                                ots.append(
                                    outp.tile([P, S], f16, tag="ot", name=f"ot{b}_{e}")
                                )
                        box[0] = psum.tile(
                            [P, TGW], f32, tag="fill", name=f"ps2_{b}_{ec}_0"
                        )
                        nc.tensor.matmul(
                            box[0],
                            wo_sb[:, ec * P : (ec + 1) * P],
                            ctxT_sb[:, b * S : b * S + TGW],
                            start=True,
                            stop=True,
                        )
                    yield mm
                    def cast(ec=ec, box=box):
                        nc.vector.tensor_copy(ots[ec][:, 0:TGW], box[0][:])
                    yield cast

            def run_outproj_tail(b, ots):
                """Output projection tail, tg-outer so only the final 8
                matmuls wait on the last eg's normalization. PSUM rotates
                through the idle qk+fill rings (4 in flight: no matmul<->cast
                ping-pong, PE stays busy enough to hold full HAM rate); casts
                alternate ScalarE/DVE; writes go in two [128,1024] waves
                (2KB lines) so the drain overlaps the remaining casts."""
                idx = 0
                for tg in range(1, NTG_B):
                    t0 = b * S + tg * TGW
                    for ec in range(NEC):
                        ps2 = psum.tile(
                            [P, TGW], f32,
                            tag=("qk" if idx % 2 == 0 else "fill"),
                            name=f"ps2_{b}_{ec}_{tg}",
                        )
                        idx += 1
                        nc.tensor.matmul(
                            ps2,
                            wo_sb[:, ec * P : (ec + 1) * P],
                            ctxT_sb[:, t0 : t0 + TGW],
                            start=True,
                            stop=True,
                        )
                        dst = ots[ec][:, tg * TGW : (tg + 1) * TGW]
                        if idx % 2 == 0:
                            nc.scalar.activation(dst, ps2[:], COPY)
                        else:
                            nc.vector.tensor_copy(dst, ps2[:])
                    if tg == 1:
                        for ec in range(NEC):
                            nc.sync.dma_start(
                                outT[ec * P : (ec + 1) * P, b * S : b * S + 2 * TGW],
                                ots[ec][:, 0 : 2 * TGW],
                            )
                for ec in range(NEC):
                    nc.sync.dma_start(
                        outT[ec * P : (ec + 1) * P, b * S + 2 * TGW : b * S + 4 * TGW],
                        ots[ec][:, 2 * TGW : 4 * TGW],
                    )

            def vaug_step_filler(b, extra, extra_rate=1):
                """Per-step filler batches for one batch's attention: the
                V-relayout chunks 2..15 (3 ops each, completing 2 steps ahead
                of the AV that reads each chunk), interleaved with `extra`
                ops (one per step), then the remaining extras 2 per step.
                Returns (preblock_ops, step_batches)."""
                vops = list(vaug_ops(b))
                pre, vops = vops[:6], vops[6:]
                extra = list(extra)
                steps = []
                for c in range(14):
                    batch = vops[c * 3 : (c + 1) * 3]
                    for _ in range(min(extra_rate, len(extra))):
                        batch.append(extra.pop(0))
                    steps.append(batch)
                while extra:
                    steps.append([extra.pop(0) for _ in range(min(2, len(extra)))])
                return pre, steps

            # ---- head: k,v,q loads; K/V projections hide in the DMA
            # ---- shadow; Q's first token group gates attention(0).
            xts0 = issue_x_loads(0, ("k", "v", "q"), with_weights=True)
            nc.vector.memset(ones_cols, 1.0)
            # preload the exp activation table set (~2.7us) during the
            # DMA-bound head instead of at the first real exp.
            exp_dummy = singles.tile([P, 1], f16)
            nc.scalar.activation(exp_dummy[:], ones_cols[:, 0, :], EXP)
            for h in range(hpc):
                one_col = h * (dk + 1) + dk
                nc.vector.tensor_copy(vaug_sb[:, :, one_col : one_col + 1], ones_cols[:])

            run_proj_one(0, "k", xts0)
            run_proj_one(0, "v", xts0)
            run_proj_one(0, "q", xts0)
            # attention(0) filler: V-relayout chunks (2 preblocked, rest 3
            # ops/step just ahead of their AVs), Q tg1-3 chains (each done
            # ahead of the eg that reads it), and the x1 DMA batch at eg0's
            # end. proj(1) stays AFTER attention(0) in program order: the
            # static scheduler weaves its chains into remaining slack as x1
            # tiles land.
            pre0, steps0 = vaug_step_filler(0, [], extra_rate=0)
            # x1's tile allocations reuse x0's 16 slots, so each x1 DMA
            # waits until the x0 tiles it replaces have been fully read by
            # every ALREADY-EMITTED reader: x1 never steals HBM bandwidth
            # from x0-q, whose landing gates the first exp. Only x1-k (whose
            # slots' readers, the x0-v chains, are all emitted) may issue
            # before attention(0); x1-v must follow the woven q0 chains and
            # x1-q must follow the k1 projection, else the DMA lands before
            # readers that are emitted later in program order (silent
            # use-after-free -- the dependency tracker only orders against
            # prior program points).
            xts1 = issue_x_loads(1, ("k", "q"))
            xts1.update(issue_x_loads(1, ("v",)))
            for op in pre0:
                op()
            run_attention(0, egs=range(NEG), filler=steps0)
            run_proj_one(1, "k", xts1)
            run_proj_one(1, "q", xts1)
            run_proj_one(1, "v", xts1)
            # k1/v1 projections and q1's first token group are emitted
            # here (the scheduler weaves them into attention(0)'s remaining
            # slack as the x1 tiles land); q1 tg1-3 weave into attention(1)
            # itself, each finishing ahead of the eg that reads it.
            ots1 = []
            ext1 = list(outproj_fill_ops(0))
            ext1.extend(outproj_tail_tg0_ops(1, ots1))
            # attention(1) filler: batch 1's V-relayout first (its AVs need
            # it chunk by chunk), then batch 0's output projection and the
            # tail batch's first token group.
            pre1, steps1 = vaug_step_filler(1, ext1, extra_rate=2)
            for op in pre1:
                op()
            run_attention(1, egs=range(NEG), filler=steps1)
            run_outproj_tail(1, ots1)

    nc.compile()
    return nc


_NC_CACHE = {}


def _compiled():
    if "nc" not in _NC_CACHE:
        _NC_CACHE["nc"] = build_program()
    return _NC_CACHE["nc"]


def _permute_w(w):
    """[D, oc] -> [p, dc, oc] so the device can fetch it with one DMA."""
    D, oc = w.shape
    return np.ascontiguousarray(w.reshape(D // 128, 128, oc).transpose(1, 0, 2))


def make_in_maps(q, k, v, Wq, bq, Wk, bk, Wv, bv, Wo):
    """Shard inputs for the 8 cores: shared transposed activations plus
    per-core head-slice weight columns / Wo rows."""
    f = np.float32
    h16 = np.float16
    qT = np.ascontiguousarray(q.transpose(2, 1, 0).reshape(D_MODEL, -1)).astype(h16)
    kT = np.ascontiguousarray(k.transpose(2, 1, 0).reshape(D_MODEL, -1)).astype(h16)
    vT = np.ascontiguousarray(v.transpose(2, 1, 0).reshape(D_MODEL, -1)).astype(h16)
    Wq, Wk, Wv, Wo = (np.asarray(w).astype(h16) for w in (Wq, Wk, Wv, Wo))
    in_maps = []
    for c in range(N_CORES):
        sl = slice(c * OC, (c + 1) * OC)
        in_maps.append(
            {
                "qT": qT,
                "kT": kT,
                "vT": vT,
                "Wq_s": _permute_w(Wq[:, sl]),
                "Wk_s": _permute_w(Wk[:, sl]),
                "Wv_s": _permute_w(Wv[:, sl]),
                "bq_s": np.ascontiguousarray(bq[sl].reshape(OC, 1), dtype=f),
                "bk_s": np.ascontiguousarray(bk[sl].reshape(OC, 1), dtype=f),
                "bv_s": np.ascontiguousarray(bv[sl].reshape(OC, 1), dtype=f),
                "Wo_s": np.ascontiguousarray(Wo[sl, :]),
            }
        )
    return in_maps


def _install_ntff_shim():
    """Provide antenv.axon_hooks (absent on some images) so that
    trace=True / BASS_TRACE=1 in run_bass_kernel_spmd works instead of
    crashing with ModuleNotFoundError. Best-effort."""
    import sys
    import types

    try:
        import antenv
    except ImportError:
        return
    try:
        import antenv.axon_hooks  # noqa: F401

        return  # real module exists
    except ImportError:
        pass
    mod = types.ModuleType("antenv.axon_hooks")
    mod._hook = None
    mod.set_axon_ntff_profile_hook = lambda h: setattr(mod, "_hook", h)
    mod.get_axon_ntff_profile_hook = lambda: mod._hook
    sys.modules["antenv.axon_hooks"] = mod
    antenv.axon_hooks = mod
    try:
        from trn_agent_boot.trn_boot import _ntff_profile_via_ctypes

        hook = _ntff_profile_via_ctypes("/opt/axon/libaxon_pjrt.so")
        if hook is not None:
            mod.set_axon_ntff_profile_hook(hook)
    except Exception:  # noqa: BLE001
        pass


def kernel(q, k, v, Wq, bq, Wk, bk, Wv, bv, Wo, bo):
    global LAST_RESULTS
    from concourse.bass_utils import run_bass_kernel_spmd

    _install_ntff_shim()

    nc = _compiled()
    in_maps = make_in_maps(q, k, v, Wq, bq, Wk, bk, Wv, bv, Wo)
    res = run_bass_kernel_spmd(nc, in_maps, core_ids=list(range(N_CORES)))
    LAST_RESULTS = res
    total = res.results[0]["outT"].astype(np.float64)
    for c in range(1, N_CORES):
        total += res.results[c]["outT"]
    out = total.reshape(D_MODEL, BATCH, SEQ).transpose(2, 1, 0) + np.asarray(
        bo, dtype=np.float64
    )
    return np.ascontiguousarray(out, dtype=np.float32)


# revision 65
# speedup vs baseline: 1.1650x; 1.1611x over previous
"""Multi-head attention (S=2048, B=2, D=1024, H=16) on 8 Trainium2 NeuronCores.

Sharding: batch*head parallel. Core c owns heads [2c, 2c+1]: it holds the
128-column slice of Wq/Wk/Wv and the matching 128-row slice of Wo, computes
its heads' attention over all tokens, and produces a partial output
projection. Partials are summed on the host (the all-reduce step).

Device layout: activations live transposed (features on partitions, tokens
on the free axis) the whole way through:
  - projections:  QT/KT/VT[oc, t] = W_slice.T @ xT        (oc = head-slice col)
  - scores (transposed): sT[j, i]  = KT_j.T @ QT           (j = key pos chunk)
  - softmax: exp on ScalarE over [128, 1024] tiles (both heads at once to
    amortize the ACT fixed overhead); the row-sum over j comes from an extra
    all-ones column appended to V in the AV matmul; normalization by 1/sum is
    a fast-approx reciprocal + GpSimd partition-broadcast + vector multiply.
  - AV:  ctxT[dk, i] (+ sum row) = [V | 1].T @ exp(sT)     (V in natural [j, dk])
  - output: outT[e, t] = Wo_slice.T @ ctxT

The middle ~140us is ScalarE-paced: ACT is a fixed 1 elem/cycle/lane spline
engine ((N+352)/1.2 ns), DVE has no exp, and PSUM capacity (8 x 2KB banks;
TRN2 matmul output must be fp32) caps the exp tile at [128,1024]. So the
whole program is built around keeping the ~1.11us/j-step exp stream
stall-free and hiding everything else in its ~360ns/step tensor slack:
  - Per-engine instruction streams are IN ORDER at runtime and program order
    is the static scheduler's priority. Filler work (other-batch projections,
    output projection) must be either left after the attention loops (the
    scheduler weaves it into idle slots) or woven explicitly 1-2 ops per
    j-step; a contiguous block placed mid-stream stalls the exp pipe, and
    filler that waits on a DMA head-of-line blocks attention.
  - x loads run k,v,q: attention needs K fully and Q's first token group, so
    Q gates the start either way, while V landing second lets V's projection
    + the V-transpose relayout run in the DMA shadow instead of eating
    attention(0)'s slack. Projections are per-token-group chains (8 matmuls
    + bias drain) so Q tg1-3 chains weave as leading filler.
  - eg boundaries: ctx PSUM banks drain to SBUF right after the last AV (two
    vector copies; the sum row goes to a partition-0 tile --
    reciprocal_approx_fast silently corrupts on partition-offset inputs),
    freeing the bank ~2us before the reciprocal/broadcast/multiply chain
    finishes.
Output is staged per 128-row chunk into [128, 2048] SBUF tiles and written
with wide-line DMAs (narrow 1KB-line DMAs measured ~100GB/s); the tail batch
goes tg-outer (its last eg's norm gates only the final 8 matmuls), rotates
PSUM tiles through the then-idle qk ring, alternates ScalarE/DVE casts, and
writes in two waves so the drain overlaps the casts.

All matmuls run in float16 operands (fp32 PSUM accumulation).
"""

import math

import numpy as np

SEQ, BATCH, D_MODEL, HEADS = 2048, 2, 1024, 16
D_K = D_MODEL // HEADS  # 64
N_CORES = 8
HPC = HEADS // N_CORES  # heads per core: 2
OC = HPC * D_K  # per-core head-slice width: 128

LAST_RESULTS = None  # BassKernelResults of the most recent kernel() call


def build_program(S=SEQ, B=BATCH, D=D_MODEL, H=HEADS, n_cores=N_CORES):
    """Build + compile the per-core bass program (SPMD: same program on all
    cores, per-core weight slices arrive via the input maps)."""
    import concourse.bass as bass  # noqa: F401
    import concourse.mybir as mybir
    import concourse.tile as tile
    from concourse import bacc
    from concourse.masks import make_identity

    dk = D // H
    hpc = H // n_cores
    oc = hpc * dk
    T = B * S
    P = 128
    assert oc == P, "kernel assumes a 128-wide per-core head slice"
    assert hpc == 2
    scale = 1.0 / math.sqrt(dk)

    NDC = D // P  # contraction chunks for the projections
    NJC = S // P  # key-position chunks per (batch, head)
    TGW = 512  # token-group width (projections / output)
    NTG_B = S // TGW  # token groups per batch
    EW = 512  # attention i-group width
    NEG = S // EW
    NEC = D // P  # output-projection column chunks

    f32 = mybir.dt.float32
    f16 = mybir.dt.float16
    EXP = mybir.ActivationFunctionType.Exp
    COPY = mybir.ActivationFunctionType.Copy

    nc = bacc.Bacc("TRN2", target_bir_lowering=False, debug=False)

    qT = nc.dram_tensor("qT", (D, T), f16, kind="ExternalInput")
    kT = nc.dram_tensor("kT", (D, T), f16, kind="ExternalInput")
    vT = nc.dram_tensor("vT", (D, T), f16, kind="ExternalInput")
    # weights arrive host-pre-permuted as [p, dc, oc] so one DMA fills the
    # SBUF layout (the sync engine's 620ns per-dma_start issue cost gates
    # the kernel head).
    Wq_s = nc.dram_tensor("Wq_s", (P, NDC, oc), f16, kind="ExternalInput")
    Wk_s = nc.dram_tensor("Wk_s", (P, NDC, oc), f16, kind="ExternalInput")
    Wv_s = nc.dram_tensor("Wv_s", (P, NDC, oc), f16, kind="ExternalInput")
    bq_s = nc.dram_tensor("bq_s", (oc, 1), f32, kind="ExternalInput")
    bk_s = nc.dram_tensor("bk_s", (oc, 1), f32, kind="ExternalInput")
    bv_s = nc.dram_tensor("bv_s", (oc, 1), f32, kind="ExternalInput")
    Wo_s = nc.dram_tensor("Wo_s", (oc, D), f16, kind="ExternalInput")
    outT = nc.dram_tensor("outT", (D, T), f16, kind="ExternalOutput")

    with tile.TileContext(nc) as tc:
        with (
            tc.tile_pool(name="singles", bufs=1) as singles,
            tc.tile_pool(name="xpool", bufs=16) as xpool,
            tc.tile_pool(name="xpoolv", bufs=8) as xpoolv,
            tc.tile_pool(name="expool", bufs=4) as expool,
            tc.tile_pool(name="small", bufs=4) as small,
            tc.tile_pool(name="outp", bufs=8) as outp,
            # PSUM (8 banks): tag "qk" = [128,1024]f32 (2 banks) x 2 bufs,
            # attention-exclusive (the output-projection tail reuses it);
            # tag "ctx" = [65,512]f32 (1 bank) x 2 bufs; tag "fill" =
            # [128,512]f32 (1 bank) x 2 bufs for proj chains / V transposes /
            # outproj so those phases never block attention.
            tc.tile_pool(name="psum", bufs=2, space=bass.MemorySpace.PSUM) as psum,
        ):
            ident = singles.tile([P, P], f16)
            make_identity(nc, ident)

            # HAM warm-up: back-to-back throwaway matmuls seed the PE's
            # activity monitor (the 4/8 clock gate wants ~3.4us of sustained
            # work) before the DMA-paced projection phase begins.
            warm_ps = psum.tile([P, 2 * EW], f32, tag="qk", name="warm_ps")
            for i in range(10):
                r = (i % 8) * P
                nc.tensor.matmul(
                    warm_ps[:, r : r + P], ident[:], ident[:], start=True, stop=True
                )

            wq_sb = singles.tile([P, NDC, oc], f16)
            wk_sb = singles.tile([P, NDC, oc], f16)
            wv_sb = singles.tile([P, NDC, oc], f16)
            wo_sb = singles.tile([oc, D], f16)
            bq_sb = singles.tile([oc, 1], f32)
            bk_sb = singles.tile([oc, 1], f32)
            bv_sb = singles.tile([oc, 1], f32)

            QT_sb = singles.tile([oc, T], f16)
            KT_sb = singles.tile([oc, T], f16)
            VT_sb = singles.tile([oc, T], f16)
            ctxT_sb = singles.tile([oc, T], f16)
            # [V | ones] stationary operands: per (batch, j-chunk) a
            # [128 tokens, hpc*(dk+1)] block, head h at cols h*(dk+1).
            vaug_sb = singles.tile([P, B * NJC, hpc * (dk + 1)], f16)
            ones_cols = singles.tile([P, B * NJC, 1], f32)

            PROJ_TBL = {
                "k": (lambda: (wk_sb, bk_sb, KT_sb)),
                "q": (lambda: (wq_sb, bq_sb, QT_sb)),
                "v": (lambda: (wv_sb, bv_sb, VT_sb)),
            }
            W_DRAM = {"k": (Wk_s, bk_s), "q": (Wq_s, bq_s), "v": (Wv_s, bv_s)}
            W_SB = {"k": (wk_sb, bk_sb), "q": (wq_sb, bq_sb), "v": (wv_sb, bv_sb)}

            def issue_x_loads(b, order, with_weights=False):
                """Issue one batch's input DMAs (8 tiles per tensor; one big
                DMA per tensor would land on too few DMA queues and lose the
                ~420GB/s aggregate). For batch 0 each tensor's weight issues
                just before its x tiles."""
                xts = {}
                for name in order:
                    if with_weights:
                        # weights issue from GPSIMD's DGE path: it fires
                        # immediately (fresh tiles, no slot waits), keeping
                        # 7 x 620ns of issue cost off the sync queue so the
                        # x-tile DMAs start ~4us earlier
                        w_sb, b_sb = W_SB[name]
                        w_dram, b_dram = W_DRAM[name]
                        nc.gpsimd.dma_start(w_sb[:, :, :], w_dram[:, :, :])
                        nc.gpsimd.dma_start(b_sb, b_dram[:, :])
                    x_dram = {"k": kT, "q": qT, "v": vT}[name]
                    pool = xpoolv if (b == 1 and name == "v") else xpool
                    tiles = []
                    for dc in range(NDC):
                        xt = pool.tile([P, S], f16, tag="xt", name=f"xt_{name}{b}_{dc}")
                        nc.sync.dma_start(
                            xt, x_dram[dc * P : (dc + 1) * P, b * S : (b + 1) * S]
                        )
                        tiles.append(xt)
                    xts[name] = tiles
                if with_weights:
                    nc.gpsimd.dma_start(wo_sb, Wo_s[:, :])
                return xts

            def proj_tg_ops(b, name, tg, xts):
                """Thunks for one token group of one projection: an 8-matmul
                dc chain into a single fill-ring PSUM tile, then a bias-add
                drain. One instruction per thunk so the chain can weave into
                the attention j-loop; the PSUM tile allocates lazily at emit
                time so ring slots are claimed in true program order."""
                tiles = xts[name]
                box = {}
                for dc in range(NDC):
                    def mm(dc=dc, box=box, name=name, tg=tg):
                        w_sb = PROJ_TBL[name]()[0]
                        if "ps" not in box:
                            box["ps"] = psum.tile(
                                [oc, TGW], f32, tag="fill", name=f"ps_{name}{b}_{tg}"
                            )
                        nc.tensor.matmul(
                            box["ps"],
                            w_sb[:, dc, :],
                            tiles[dc][:, tg * TGW : (tg + 1) * TGW],
                            start=(dc == 0),
                            stop=(dc == NDC - 1),
                        )
                    yield mm
                def drain(box=box, name=name, tg=tg):
                    b_sb, dstT = PROJ_TBL[name]()[1:]
                    t0 = b * S + tg * TGW
                    nc.vector.tensor_scalar_add(dstT[:, t0 : t0 + TGW], box["ps"], b_sb[:])
                yield drain

            def run_proj_one(b, name, xts, tgs=None):
                for tg in tgs if tgs is not None else range(NTG_B):
                    for op in proj_tg_ops(b, name, tg, xts):
                        op()

            def vaug_ops(b):
                """Thunks for V's natural layout (+ ones cols): PE transpose
                then two strided head copies per j-chunk."""
                for j in range(NJC):
                    pst_box = {}
                    def tr(j=j, pst_box=pst_box):
                        pst_box[0] = psum.tile([P, P], f16, tag="fill", name=f"pst{b}_{j}")
                        nc.tensor.transpose(
                            pst_box[0],
                            VT_sb[:, b * S + j * P : b * S + (j + 1) * P],
                            ident[:],
                        )
                    yield tr
                    for h in range(hpc):
                        def cp(j=j, h=h, pst_box=pst_box):
                            nc.vector.tensor_copy(
                                vaug_sb[:, b * NJC + j, h * (dk + 1) : h * (dk + 1) + dk],
                                pst_box[0][:, h * dk : (h + 1) * dk],
                            )
                        yield cp

            def run_attention(b, egs, filler=None, skip_steps=0):
                """Attention for both heads of batch b. Software-pipelined:
                QK(j+1) issues before AV(j) so exp(j) (ScalarE) is complete by
                the time the tensor engine reaches AV(j). The two heads' K=64
                QK matmuls land in disjoint PE row groups (h0 rows 0-63, h1
                rows 64-127) writing halves of one 1024-wide PSUM tile, and
                run concurrently; one 1024-wide exp covers both heads.
                `filler` is a list of per-step op batches; one batch is
                emitted after each j-step (the slack fits ~1.7 matmuls per
                step -- batches bigger than that stretch the step)."""
                filler = list(filler) if filler else []
                fi = 0
                step = 0
                QTp = [QT_sb[h * dk : (h + 1) * dk, b * S : (b + 1) * S] for h in range(hpc)]
                KTp = [KT_sb[h * dk : (h + 1) * dk, b * S : (b + 1) * S] for h in range(hpc)]
                for eg in egs:
                    i0 = eg * EW
                    ctxs = [
                        psum.tile(
                            [dk + 1, EW], f32, tag="ctx", bufs=2, name=f"ctx{b}_{eg}_{h}"
                        )
                        for h in range(hpc)
                    ]

                    def issue_qk(j):
                        qk2 = psum.tile([P, 2 * EW], f32, tag="qk", name=f"qk{b}_{eg}_{j}")
                        for h in range(hpc):
                            nc.tensor.matmul(
                                qk2[:, h * EW : (h + 1) * EW],
                                KTp[h][:, j * P : (j + 1) * P],
                                QTp[h][:, i0 : i0 + EW],
                                start=True,
                                stop=True,
                            )
                        return qk2

                    def issue_exp_av(j, qk2):
                        ex2 = expool.tile([P, 2 * EW], f16, tag="ex", name="ex2")
                        nc.scalar.activation(ex2[:], qk2[:], EXP, scale=scale)
                        for h in range(hpc):
                            nc.tensor.matmul(
                                ctxs[h],
                                vaug_sb[:, b * NJC + j, h * (dk + 1) : (h + 1) * (dk + 1)],
                                ex2[:, h * EW : (h + 1) * EW],
                                start=(j == 0),
                                stop=(j == NJC - 1),
                            )

                    prev = issue_qk(0)
                    for j in range(1, NJC):
                        cur = issue_qk(j)
                        issue_exp_av(j - 1, prev)
                        prev = cur
                        step += 1
                        if step > skip_steps and fi < len(filler):
                            for op in filler[fi]:
                                op()
                            fi += 1
                    issue_exp_av(NJC - 1, prev)

                    # Drain both ctx PSUM banks to SBUF immediately (frees
                    # them for the next eg's AV chain ~2us sooner than
                    # normalizing from PSUM), then normalize from SBUF.
                    csb = []
                    for h in range(hpc):
                        c = small.tile([dk, EW], f32, tag="ctxsb", bufs=4,
                                       name=f"csb{b}_{eg}_{h}")
                        sr = small.tile([1, EW], f32, tag="srow", bufs=2,
                                        name=f"sr{b}_{eg}_{h}")
                        nc.vector.tensor_copy(c[:], ctxs[h][0:dk, :])
                        nc.vector.tensor_copy(sr[:], ctxs[h][dk : dk + 1, :])
                        csb.append((c, sr))
                    for h in range(hpc):
                        c, sr = csb[h]
                        rec = small.tile([1, EW], f32, tag="rec", bufs=2)
                        nc.vector.reciprocal_approx_fast(rec[:], sr[:])
                        bc = small.tile([dk, EW], f32, tag="bc", bufs=2)
                        nc.gpsimd.partition_broadcast(bc[:], rec[:])
                        nc.vector.tensor_mul(
                            ctxT_sb[h * dk : (h + 1) * dk, b * S + i0 : b * S + i0 + EW],
                            c[:],
                            bc[:],
                        )
                # leftover filler runs here (a block: only safe at the end
                # of a batch where the next phase tolerates a short stall)
                while fi < len(filler):
                    for op in filler[fi]:
                        op()
                    fi += 1

            def outproj_fill_ops(b):
                """Thunks for the ready batch's output projection, woven into
                the other batch's attention. ec-outer: each 128-row chunk
                accumulates 4 token-group casts into one [128, 2048] SBUF
                tile, then one DMA (4KB lines). Casts on DVE (ScalarE is the
                exp bottleneck)."""
                for ec in range(NEC):
                    box = {}
                    for tg in range(NTG_B):
                        def mm(ec=ec, tg=tg, box=box):
                            if "ot" not in box:
                                box["ot"] = outp.tile([P, S], f16, tag="ot", name=f"ot{b}_{ec}")
                            t0 = b * S + tg * TGW
                            box[tg] = psum.tile(
                                [P, TGW], f32, tag="fill", name=f"ps2_{b}_{ec}_{tg}"
                            )
                            nc.tensor.matmul(
                                box[tg],
                                wo_sb[:, ec * P : (ec + 1) * P],
                                ctxT_sb[:, t0 : t0 + TGW],
                                start=True,
                                stop=True,
                            )
                        yield mm
                        def cast(ec=ec, tg=tg, box=box):
                            nc.vector.tensor_copy(
                                box["ot"][:, tg * TGW : (tg + 1) * TGW], box[tg][:]
                            )
                        yield cast
                    def wr(ec=ec, box=box):
                        nc.sync.dma_start(
                            outT[ec * P : (ec + 1) * P, b * S : (b + 1) * S], box["ot"][:]
                        )
                    yield wr

            def outproj_tail_tg0_ops(b, ots):
                """Thunks for the tail batch's first token group (8 matmuls +
                DVE casts), woven into the tail end of the other-batch
                attention: its ctxT is ready after eg0 and the fill ring is
                free once the fill-batch output projection has drained, so
                only token groups 1-3 remain after the last exp."""
                for ec in range(NEC):
                    box = {}
                    def mm(ec=ec, box=box):
                        # ots allocate here, AFTER the fill batch's 8 "ot"
                        # allocations: these 8 live until the final waves, so
                        # allocating them first would deadlock the 8-slot ring
                        if not ots:
                            for e in range(NEC):
                                ots.append(
                                    outp.tile([P, S], f16, tag="ot", name=f"ot{b}_{e}")
                                )
                        box[0] = psum.tile(
                            [P, TGW], f32, tag="fill", name=f"ps2_{b}_{ec}_0"
                        )
                        nc.tensor.matmul(
                            box[0],
                            wo_sb[:, ec * P : (ec + 1) * P],
                            ctxT_sb[:, b * S : b * S + TGW],
                            start=True,
                            stop=True,
                        )
                    yield mm
                    def cast(ec=ec, box=box):
                        nc.vector.tensor_copy(ots[ec][:, 0:TGW], box[0][:])
                    yield cast

            def run_outproj_tail(b, ots):
                """Output projection tail, tg-outer so only the final 8
                matmuls wait on the last eg's normalization. PSUM rotates
                through the idle qk+fill rings (4 in flight: no matmul<->cast
                ping-pong, PE stays busy enough to hold full HAM rate); casts
                alternate ScalarE/DVE; writes go in two [128,1024] waves
                (2KB lines) so the drain overlaps the remaining casts."""
                idx = 0
                for tg in range(1, NTG_B):
                    t0 = b * S + tg * TGW
                    for ec in range(NEC):
                        ps2 = psum.tile(
                            [P, TGW], f32,
                            tag=("qk" if idx % 2 == 0 else "fill"),
                            name=f"ps2_{b}_{ec}_{tg}",
                        )
                        idx += 1
                        nc.tensor.matmul(
                            ps2,
                            wo_sb[:, ec * P : (ec + 1) * P],
                            ctxT_sb[:, t0 : t0 + TGW],
                            start=True,
                            stop=True,
                        )
                        dst = ots[ec][:, tg * TGW : (tg + 1) * TGW]
                        if idx % 2 == 0:
                            nc.scalar.activation(dst, ps2[:], COPY)
                        else:
                            nc.vector.tensor_copy(dst, ps2[:])
                    if tg == 1:
                        # GPSIMD's DGE path fires each wave DMA the moment
                        # its casts' sems clear; on the sync queue the 8
                        # issues would serialize at 620ns each
                        for ec in range(NEC):
                            nc.gpsimd.dma_start(
                                outT[ec * P : (ec + 1) * P, b * S : b * S + 2 * TGW],
                                ots[ec][:, 0 : 2 * TGW],
                            )
                for ec in range(NEC):
                    nc.gpsimd.dma_start(
                        outT[ec * P : (ec + 1) * P, b * S + 2 * TGW : b * S + 4 * TGW],
                        ots[ec][:, 2 * TGW : 4 * TGW],
                    )

            def vaug_step_filler(b, extra, extra_rate=1):
                """Per-step filler batches for one batch's attention: the
                V-relayout chunks 2..15 (3 ops each, completing 2 steps ahead
                of the AV that reads each chunk), interleaved with `extra`
                ops (one per step), then the remaining extras 2 per step.
                Returns (preblock_ops, step_batches)."""
                vops = list(vaug_ops(b))
                pre, vops = vops[:6], vops[6:]
                extra = list(extra)
                steps = []
                for c in range(14):
                    batch = vops[c * 3 : (c + 1) * 3]
                    for _ in range(min(extra_rate, len(extra))):
                        batch.append(extra.pop(0))
                    steps.append(batch)
                while extra:
                    steps.append([extra.pop(0) for _ in range(min(2, len(extra)))])
                return pre, steps

            # ---- head: k,v,q loads; K/V projections hide in the DMA
            # ---- shadow; Q's first token group gates attention(0).
            xts0 = issue_x_loads(0, ("k", "v", "q"), with_weights=True)
            nc.vector.memset(ones_cols, 1.0)
            # preload the exp activation table set (~2.7us) during the
            # DMA-bound head instead of at the first real exp.
            exp_dummy = singles.tile([P, 1], f16)
            nc.scalar.activation(exp_dummy[:], ones_cols[:, 0, :], EXP)
            for h in range(hpc):
                one_col = h * (dk + 1) + dk
                nc.vector.tensor_copy(vaug_sb[:, :, one_col : one_col + 1], ones_cols[:])

            run_proj_one(0, "k", xts0)
            run_proj_one(0, "v", xts0)
            run_proj_one(0, "q", xts0)
            # attention(0) filler: V-relayout chunks (2 preblocked, rest 3
            # ops/step just ahead of their AVs), Q tg1-3 chains (each done
            # ahead of the eg that reads it), and the x1 DMA batch at eg0's
            # end. proj(1) stays AFTER attention(0) in program order: the
            # static scheduler weaves its chains into remaining slack as x1
            # tiles land.
            pre0, steps0 = vaug_step_filler(0, [], extra_rate=0)
            # x1's tile allocations reuse x0's 16 slots, so each x1 DMA
            # waits until the x0 tiles it replaces have been fully read by
            # every ALREADY-EMITTED reader: x1 never steals HBM bandwidth
            # from x0-q, whose landing gates the first exp. Only x1-k (whose
            # slots' readers, the x0-v chains, are all emitted) may issue
            # before attention(0); x1-v must follow the woven q0 chains and
            # x1-q must follow the k1 projection, else the DMA lands before
            # readers that are emitted later in program order (silent
            # use-after-free -- the dependency tracker only orders against
            # prior program points).
            xts1 = issue_x_loads(1, ("k", "q"))
            xts1.update(issue_x_loads(1, ("v",)))
            for op in pre0:
                op()
            run_attention(0, egs=range(NEG), filler=steps0)
            run_proj_one(1, "k", xts1)
            run_proj_one(1, "q", xts1)
            run_proj_one(1, "v", xts1)
            # k1/v1 projections and q1's first token group are emitted
            # here (the scheduler weaves them into attention(0)'s remaining
            # slack as the x1 tiles land); q1 tg1-3 weave into attention(1)
            # itself, each finishing ahead of the eg that reads it.
            ots1 = []
            ext1 = list(outproj_fill_ops(0))
            ext1.extend(outproj_tail_tg0_ops(1, ots1))
            # attention(1) filler: batch 1's V-relayout first (its AVs need
            # it chunk by chunk), then batch 0's output projection and the
            # tail batch's first token group.
            pre1, steps1 = vaug_step_filler(1, ext1, extra_rate=2)
            for op in pre1:
                op()
            run_attention(1, egs=range(NEG), filler=steps1)
            run_outproj_tail(1, ots1)

    nc.compile()
    return nc


_NC_CACHE = {}


def _compiled():
    if "nc" not in _NC_CACHE:
        _NC_CACHE["nc"] = build_program()
    return _NC_CACHE["nc"]


def _permute_w(w):
    """[D, oc] -> [p, dc, oc] so the device can fetch it with one DMA."""
    D, oc = w.shape
    return np.ascontiguousarray(w.reshape(D // 128, 128, oc).transpose(1, 0, 2))


def make_in_maps(q, k, v, Wq, bq, Wk, bk, Wv, bv, Wo):
    """Shard inputs for the 8 cores: shared transposed activations plus
    per-core head-slice weight columns / Wo rows."""
    f = np.float32
    h16 = np.float16
    qT = np.ascontiguousarray(q.transpose(2, 1, 0).reshape(D_MODEL, -1)).astype(h16)
    kT = np.ascontiguousarray(k.transpose(2, 1, 0).reshape(D_MODEL, -1)).astype(h16)
    vT = np.ascontiguousarray(v.transpose(2, 1, 0).reshape(D_MODEL, -1)).astype(h16)
    Wq, Wk, Wv, Wo = (np.asarray(w).astype(h16) for w in (Wq, Wk, Wv, Wo))
    in_maps = []
    for c in range(N_CORES):
        sl = slice(c * OC, (c + 1) * OC)
        in_maps.append(
            {
                "qT": qT,
                "kT": kT,
                "vT": vT,
                "Wq_s": _permute_w(Wq[:, sl]),
                "Wk_s": _permute_w(Wk[:, sl]),
                "Wv_s": _permute_w(Wv[:, sl]),
                "bq_s": np.ascontiguousarray(bq[sl].reshape(OC, 1), dtype=f),
                "bk_s": np.ascontiguousarray(bk[sl].reshape(OC, 1), dtype=f),
                "bv_s": np.ascontiguousarray(bv[sl].reshape(OC, 1), dtype=f),
                "Wo_s": np.ascontiguousarray(Wo[sl, :]),
            }
        )
    return in_maps


def _install_ntff_shim():
    """Provide antenv.axon_hooks (absent on some images) so that
    trace=True / BASS_TRACE=1 in run_bass_kernel_spmd works instead of
    crashing with ModuleNotFoundError. Best-effort."""
    import sys
    import types

    try:
        import antenv
    except ImportError:
        return
    try:
        import antenv.axon_hooks  # noqa: F401

        return  # real module exists
    except ImportError:
        pass
    mod = types.ModuleType("antenv.axon_hooks")
    mod._hook = None
    mod.set_axon_ntff_profile_hook = lambda h: setattr(mod, "_hook", h)
    mod.get_axon_ntff_profile_hook = lambda: mod._hook
    sys.modules["antenv.axon_hooks"] = mod
    antenv.axon_hooks = mod
    try:
        from trn_agent_boot.trn_boot import _ntff_profile_via_ctypes

        hook = _ntff_profile_via_ctypes("/opt/axon/libaxon_pjrt.so")
        if hook is not None:
            mod.set_axon_ntff_profile_hook(hook)
    except Exception:  # noqa: BLE001
        pass


def kernel(q, k, v, Wq, bq, Wk, bk, Wv, bv, Wo, bo):
    global LAST_RESULTS
    from concourse.bass_utils import run_bass_kernel_spmd

    _install_ntff_shim()

    nc = _compiled()
    in_maps = make_in_maps(q, k, v, Wq, bq, Wk, bk, Wv, bv, Wo)
    res = run_bass_kernel_spmd(nc, in_maps, core_ids=list(range(N_CORES)))
    LAST_RESULTS = res
    total = res.results[0]["outT"].astype(np.float64)
    for c in range(1, N_CORES):
        total += res.results[c]["outT"]
    out = total.reshape(D_MODEL, BATCH, SEQ).transpose(2, 1, 0) + np.asarray(
        bo, dtype=np.float64
    )
    return np.ascontiguousarray(out, dtype=np.float32)


# revision 66
# speedup vs baseline: 1.2063x; 1.0354x over previous
"""Multi-head attention (S=2048, B=2, D=1024, H=16) on 8 Trainium2 NeuronCores.

Sharding: batch*head parallel. Core c owns heads [2c, 2c+1]: it holds the
128-column slice of Wq/Wk/Wv and the matching 128-row slice of Wo, computes
its heads' attention over all tokens, and produces a partial output
projection. Partials are summed on the host (the all-reduce step).

Device layout: activations live transposed (features on partitions, tokens
on the free axis) the whole way through:
  - projections:  QT/KT/VT[oc, t] = W_slice.T @ xT        (oc = head-slice col)
  - scores (transposed): sT[j, i]  = KT_j.T @ QT           (j = key pos chunk)
  - softmax: exp on ScalarE over [128, 1024] tiles (both heads at once to
    amortize the ACT fixed overhead); the row-sum over j comes from an extra
    all-ones column appended to V in the AV matmul; normalization by 1/sum is
    a fast-approx reciprocal + GpSimd partition-broadcast + vector multiply.
  - AV:  ctxT[dk, i] (+ sum row) = [V | 1].T @ exp(sT)     (V in natural [j, dk])
  - output: outT[e, t] = Wo_slice.T @ ctxT

The middle ~140us is ScalarE-paced: ACT is a fixed 1 elem/cycle/lane spline
engine ((N+352)/1.2 ns), DVE has no exp, and PSUM capacity (8 x 2KB banks;
TRN2 matmul output must be fp32) caps the exp tile at [128,1024]. So the
whole program is built around keeping the ~1.11us/j-step exp stream
stall-free and hiding everything else in its ~360ns/step tensor slack:
  - Per-engine instruction streams are IN ORDER at runtime and program order
    is the static scheduler's priority. Filler work (other-batch projections,
    output projection) must be either left after the attention loops (the
    scheduler weaves it into idle slots) or woven explicitly 1-2 ops per
    j-step; a contiguous block placed mid-stream stalls the exp pipe, and
    filler that waits on a DMA head-of-line blocks attention.
  - x loads run k,v,q: attention needs K fully and Q's first token group, so
    Q gates the start either way, while V landing second lets V's projection
    + the V-transpose relayout run in the DMA shadow instead of eating
    attention(0)'s slack. Projections are per-token-group chains (8 matmuls
    + bias drain) so Q tg1-3 chains weave as leading filler.
  - eg boundaries: ctx PSUM banks drain to SBUF right after the last AV (two
    vector copies; the sum row goes to a partition-0 tile --
    reciprocal_approx_fast silently corrupts on partition-offset inputs),
    freeing the bank ~2us before the reciprocal/broadcast/multiply chain
    finishes.
Output is staged per 128-row chunk into [128, 2048] SBUF tiles and written
with wide-line DMAs (narrow 1KB-line DMAs measured ~100GB/s); the tail batch
goes tg-outer (its last eg's norm gates only the final 8 matmuls), rotates
PSUM tiles through the then-idle qk ring, alternates ScalarE/DVE casts, and
writes in two waves so the drain overlaps the casts.

All matmuls run in float16 operands (fp32 PSUM accumulation).
"""

import math

import numpy as np

SEQ, BATCH, D_MODEL, HEADS = 2048, 2, 1024, 16
D_K = D_MODEL // HEADS  # 64
N_CORES = 8
HPC = HEADS // N_CORES  # heads per core: 2
OC = HPC * D_K  # per-core head-slice width: 128

LAST_RESULTS = None  # BassKernelResults of the most recent kernel() call


def build_program(S=SEQ, B=BATCH, D=D_MODEL, H=HEADS, n_cores=N_CORES):
    """Build + compile the per-core bass program (SPMD: same program on all
    cores, per-core weight slices arrive via the input maps)."""
    import concourse.bass as bass  # noqa: F401
    import concourse.mybir as mybir
    import concourse.tile as tile
    from concourse import bacc
    from concourse.masks import make_identity

    dk = D // H
    hpc = H // n_cores
    oc = hpc * dk
    T = B * S
    P = 128
    assert oc == P, "kernel assumes a 128-wide per-core head slice"
    assert hpc == 2
    scale = 1.0 / math.sqrt(dk)

    NDC = D // P  # contraction chunks for the projections
    NJC = S // P  # key-position chunks per (batch, head)
    TGW = 512  # token-group width (projections / output)
    NTG_B = S // TGW  # token groups per batch
    EW = 512  # attention i-group width
    NEG = S // EW
    NEC = D // P  # output-projection column chunks

    f32 = mybir.dt.float32
    f16 = mybir.dt.float16
    EXP = mybir.ActivationFunctionType.Exp
    COPY = mybir.ActivationFunctionType.Copy

    nc = bacc.Bacc("TRN2", target_bir_lowering=False, debug=False)

    qT = nc.dram_tensor("qT", (D, T), f16, kind="ExternalInput")
    kT = nc.dram_tensor("kT", (D, T), f16, kind="ExternalInput")
    vT = nc.dram_tensor("vT", (D, T), f16, kind="ExternalInput")
    # weights arrive host-pre-permuted as [p, dc, oc] so one DMA fills the
    # SBUF layout (the sync engine's 620ns per-dma_start issue cost gates
    # the kernel head).
    Wq_s = nc.dram_tensor("Wq_s", (P, NDC, oc), f16, kind="ExternalInput")
    Wk_s = nc.dram_tensor("Wk_s", (P, NDC, oc), f16, kind="ExternalInput")
    Wv_s = nc.dram_tensor("Wv_s", (P, NDC, oc), f16, kind="ExternalInput")
    bq_s = nc.dram_tensor("bq_s", (oc, 1), f32, kind="ExternalInput")
    bk_s = nc.dram_tensor("bk_s", (oc, 1), f32, kind="ExternalInput")
    bv_s = nc.dram_tensor("bv_s", (oc, 1), f32, kind="ExternalInput")
    Wo_s = nc.dram_tensor("Wo_s", (oc, D), f16, kind="ExternalInput")
    outT = nc.dram_tensor("outT", (D, T), f16, kind="ExternalOutput")

    with tile.TileContext(nc) as tc:
        with (
            tc.tile_pool(name="singles", bufs=1) as singles,
            tc.tile_pool(name="xpool", bufs=16) as xpool,
            tc.tile_pool(name="xpoolv", bufs=8) as xpoolv,
            tc.tile_pool(name="expool", bufs=4) as expool,
            tc.tile_pool(name="small", bufs=4) as small,
            tc.tile_pool(name="outp", bufs=8) as outp,
            # PSUM (8 banks): tag "qk" = [128,1024]f32 (2 banks) x 2 bufs,
            # attention-exclusive (the output-projection tail reuses it);
            # tag "ctx" = [65,512]f32 (1 bank) x 2 bufs; tag "fill" =
            # [128,512]f32 (1 bank) x 2 bufs for proj chains / V transposes /
            # outproj so those phases never block attention.
            tc.tile_pool(name="psum", bufs=2, space=bass.MemorySpace.PSUM) as psum,
        ):
            ident = singles.tile([P, P], f16)
            make_identity(nc, ident)

            # HAM warm-up: back-to-back throwaway matmuls seed the PE's
            # activity monitor (the 4/8 clock gate wants ~3.4us of sustained
            # work) before the DMA-paced projection phase begins.
            warm_ps = psum.tile([P, 2 * EW], f32, tag="qk", name="warm_ps")
            for i in range(10):
                r = (i % 8) * P
                nc.tensor.matmul(
                    warm_ps[:, r : r + P], ident[:], ident[:], start=True, stop=True
                )

            wq_sb = singles.tile([P, NDC, oc], f16)
            wk_sb = singles.tile([P, NDC, oc], f16)
            wv_sb = singles.tile([P, NDC, oc], f16)
            wo_sb = singles.tile([oc, D], f16)
            bq_sb = singles.tile([oc, 1], f32)
            bk_sb = singles.tile([oc, 1], f32)
            bv_sb = singles.tile([oc, 1], f32)

            QT_sb = singles.tile([oc, T], f16)
            KT_sb = singles.tile([oc, T], f16)
            VT_sb = singles.tile([oc, T], f16)
            ctxT_sb = singles.tile([oc, T], f16)
            # [V | ones] stationary operands: per (batch, j-chunk) a
            # [128 tokens, hpc*(dk+1)] block, head h at cols h*(dk+1).
            vaug_sb = singles.tile([P, B * NJC, hpc * (dk + 1)], f16)
            ones_cols = singles.tile([P, B * NJC, 1], f32)

            PROJ_TBL = {
                "k": (lambda: (wk_sb, bk_sb, KT_sb)),
                "q": (lambda: (wq_sb, bq_sb, QT_sb)),
                "v": (lambda: (wv_sb, bv_sb, VT_sb)),
            }
            W_DRAM = {"k": (Wk_s, bk_s), "q": (Wq_s, bq_s), "v": (Wv_s, bv_s)}
            W_SB = {"k": (wk_sb, bk_sb), "q": (wq_sb, bq_sb), "v": (wv_sb, bv_sb)}

            def issue_x_loads(b, order, with_weights=False):
                """Issue one batch's input DMAs (8 tiles per tensor; one big
                DMA per tensor would land on too few DMA queues and lose the
                ~420GB/s aggregate). For batch 0 each tensor's weight issues
                just before its x tiles."""
                xts = {}
                for name in order:
                    if with_weights:
                        # weights issue from GPSIMD's DGE path: it fires
                        # immediately (fresh tiles, no slot waits), keeping
                        # 7 x 620ns of issue cost off the sync queue so the
                        # x-tile DMAs start ~4us earlier
                        w_sb, b_sb = W_SB[name]
                        w_dram, b_dram = W_DRAM[name]
                        nc.gpsimd.dma_start(w_sb[:, :, :], w_dram[:, :, :])
                        nc.gpsimd.dma_start(b_sb, b_dram[:, :])
                    x_dram = {"k": kT, "q": qT, "v": vT}[name]
                    pool = xpoolv if (b == 1 and name == "v") else xpool
                    tiles = []
                    for dc in range(NDC):
                        xt = pool.tile([P, S], f16, tag="xt", name=f"xt_{name}{b}_{dc}")
                        nc.sync.dma_start(
                            xt, x_dram[dc * P : (dc + 1) * P, b * S : (b + 1) * S]
                        )
                        tiles.append(xt)
                    xts[name] = tiles
                if with_weights:
                    nc.gpsimd.dma_start(wo_sb, Wo_s[:, :])
                return xts

            def proj_tg_ops(b, name, tg, xts):
                """Thunks for one token group of one projection: an 8-matmul
                dc chain into a single fill-ring PSUM tile, then a bias-add
                drain. One instruction per thunk so the chain can weave into
                the attention j-loop; the PSUM tile allocates lazily at emit
                time so ring slots are claimed in true program order."""
                tiles = xts[name]
                box = {}
                for dc in range(NDC):
                    def mm(dc=dc, box=box, name=name, tg=tg):
                        w_sb = PROJ_TBL[name]()[0]
                        if "ps" not in box:
                            box["ps"] = psum.tile(
                                [oc, TGW], f32, tag="fill", name=f"ps_{name}{b}_{tg}"
                            )
                        nc.tensor.matmul(
                            box["ps"],
                            w_sb[:, dc, :],
                            tiles[dc][:, tg * TGW : (tg + 1) * TGW],
                            start=(dc == 0),
                            stop=(dc == NDC - 1),
                        )
                    yield mm
                def drain(box=box, name=name, tg=tg):
                    b_sb, dstT = PROJ_TBL[name]()[1:]
                    t0 = b * S + tg * TGW
                    nc.vector.tensor_scalar_add(dstT[:, t0 : t0 + TGW], box["ps"], b_sb[:])
                yield drain

            def run_proj_one(b, name, xts, tgs=None):
                for tg in tgs if tgs is not None else range(NTG_B):
                    for op in proj_tg_ops(b, name, tg, xts):
                        op()

            def vaug_ops(b):
                """Thunks for V's natural layout (+ ones cols): PE transpose
                then two strided head copies per j-chunk."""
                for j in range(NJC):
                    pst_box = {}
                    def tr(j=j, pst_box=pst_box):
                        pst_box[0] = psum.tile([P, P], f16, tag="fill", name=f"pst{b}_{j}")
                        nc.tensor.transpose(
                            pst_box[0],
                            VT_sb[:, b * S + j * P : b * S + (j + 1) * P],
                            ident[:],
                        )
                    yield tr
                    for h in range(hpc):
                        def cp(j=j, h=h, pst_box=pst_box):
                            nc.vector.tensor_copy(
                                vaug_sb[:, b * NJC + j, h * (dk + 1) : h * (dk + 1) + dk],
                                pst_box[0][:, h * dk : (h + 1) * dk],
                            )
                        yield cp

            def run_attention(b, egs, filler=None, skip_steps=0):
                """Attention for both heads of batch b. Software-pipelined:
                QK(j+1) issues before AV(j) so exp(j) (ScalarE) is complete by
                the time the tensor engine reaches AV(j). The two heads' K=64
                QK matmuls land in disjoint PE row groups (h0 rows 0-63, h1
                rows 64-127) writing halves of one 1024-wide PSUM tile, and
                run concurrently; one 1024-wide exp covers both heads.
                `filler` is a list of per-step op batches; one batch is
                emitted after each j-step (the slack fits ~1.7 matmuls per
                step -- batches bigger than that stretch the step)."""
                filler = list(filler) if filler else []
                fi = 0
                step = 0
                QTp = [QT_sb[h * dk : (h + 1) * dk, b * S : (b + 1) * S] for h in range(hpc)]
                KTp = [KT_sb[h * dk : (h + 1) * dk, b * S : (b + 1) * S] for h in range(hpc)]
                for eg in egs:
                    i0 = eg * EW
                    ctxs = [
                        psum.tile(
                            [dk + 1, EW], f32, tag="ctx", bufs=2, name=f"ctx{b}_{eg}_{h}"
                        )
                        for h in range(hpc)
                    ]

                    def issue_qk(j):
                        qk2 = psum.tile([P, 2 * EW], f32, tag="qk", name=f"qk{b}_{eg}_{j}")
                        for h in range(hpc):
                            nc.tensor.matmul(
                                qk2[:, h * EW : (h + 1) * EW],
                                KTp[h][:, j * P : (j + 1) * P],
                                QTp[h][:, i0 : i0 + EW],
                                start=True,
                                stop=True,
                            )
                        return qk2

                    def issue_exp_av(j, qk2):
                        ex2 = expool.tile([P, 2 * EW], f16, tag="ex", name="ex2")
                        nc.scalar.activation(ex2[:], qk2[:], EXP, scale=scale)
                        for h in range(hpc):
                            nc.tensor.matmul(
                                ctxs[h],
                                vaug_sb[:, b * NJC + j, h * (dk + 1) : (h + 1) * (dk + 1)],
                                ex2[:, h * EW : (h + 1) * EW],
                                start=(j == 0),
                                stop=(j == NJC - 1),
                            )

                    prev = issue_qk(0)
                    for j in range(1, NJC):
                        cur = issue_qk(j)
                        issue_exp_av(j - 1, prev)
                        prev = cur
                        step += 1
                        if step > skip_steps and fi < len(filler):
                            for op in filler[fi]:
                                op()
                            fi += 1
                    issue_exp_av(NJC - 1, prev)

                    # Drain both ctx PSUM banks to SBUF immediately (frees
                    # them for the next eg's AV chain ~2us sooner than
                    # normalizing from PSUM), then normalize from SBUF.
                    csb = []
                    for h in range(hpc):
                        c = small.tile([dk, EW], f32, tag="ctxsb", bufs=4,
                                       name=f"csb{b}_{eg}_{h}")
                        sr = small.tile([1, EW], f32, tag="srow", bufs=2,
                                        name=f"sr{b}_{eg}_{h}")
                        nc.vector.tensor_copy(c[:], ctxs[h][0:dk, :])
                        nc.vector.tensor_copy(sr[:], ctxs[h][dk : dk + 1, :])
                        csb.append((c, sr))
                    for h in range(hpc):
                        c, sr = csb[h]
                        rec = small.tile([1, EW], f32, tag="rec", bufs=2)
                        nc.vector.reciprocal_approx_fast(rec[:], sr[:])
                        bc = small.tile([dk, EW], f32, tag="bc", bufs=2)
                        nc.gpsimd.partition_broadcast(bc[:], rec[:])
                        nc.vector.tensor_mul(
                            ctxT_sb[h * dk : (h + 1) * dk, b * S + i0 : b * S + i0 + EW],
                            c[:],
                            bc[:],
                        )
                # leftover filler runs here (a block: only safe at the end
                # of a batch where the next phase tolerates a short stall)
                while fi < len(filler):
                    for op in filler[fi]:
                        op()
                    fi += 1

            def outproj_fill_ops(b):
                """Thunks for the ready batch's output projection, woven into
                the other batch's attention. ec-outer: each 128-row chunk
                accumulates 4 token-group casts into one [128, 2048] SBUF
                tile, then one DMA (4KB lines). Casts on DVE (ScalarE is the
                exp bottleneck)."""
                for ec in range(NEC):
                    box = {}
                    for tg in range(NTG_B):
                        def mm(ec=ec, tg=tg, box=box):
                            if "ot" not in box:
                                box["ot"] = outp.tile([P, S], f16, tag="ot", name=f"ot{b}_{ec}")
                            t0 = b * S + tg * TGW
                            box[tg] = psum.tile(
                                [P, TGW], f32, tag="fill", name=f"ps2_{b}_{ec}_{tg}"
                            )
                            nc.tensor.matmul(
                                box[tg],
                                wo_sb[:, ec * P : (ec + 1) * P],
                                ctxT_sb[:, t0 : t0 + TGW],
                                start=True,
                                stop=True,
                            )
                        yield mm
                        def cast(ec=ec, tg=tg, box=box):
                            nc.vector.tensor_copy(
                                box["ot"][:, tg * TGW : (tg + 1) * TGW], box[tg][:]
                            )
                        yield cast
                    def wr(ec=ec, box=box):
                        nc.sync.dma_start(
                            outT[ec * P : (ec + 1) * P, b * S : (b + 1) * S], box["ot"][:]
                        )
                    yield wr

            def outproj_tail_tg0_ops(b, ots):
                """Thunks for the tail batch's first token group (8 matmuls +
                DVE casts), woven into the tail end of the other-batch
                attention: its ctxT is ready after eg0 and the fill ring is
                free once the fill-batch output projection has drained, so
                only token groups 1-3 remain after the last exp."""
                for ec in range(NEC):
                    box = {}
                    def mm(ec=ec, box=box):
                        # ots allocate here, AFTER the fill batch's 8 "ot"
                        # allocations: these 8 live until the final waves, so
                        # allocating them first would deadlock the 8-slot ring
                        if not ots:
                            for e in range(NEC):
                                ots.append(
                                    outp.tile([P, S], f16, tag="ot", name=f"ot{b}_{e}")
                                )
                        box[0] = psum.tile(
                            [P, TGW], f32, tag="fill", name=f"ps2_{b}_{ec}_0"
                        )
                        nc.tensor.matmul(
                            box[0],
                            wo_sb[:, ec * P : (ec + 1) * P],
                            ctxT_sb[:, b * S : b * S + TGW],
                            start=True,
                            stop=True,
                        )
                    yield mm
                    def cast(ec=ec, box=box):
                        nc.vector.tensor_copy(ots[ec][:, 0:TGW], box[0][:])
                    yield cast

            def run_outproj_tail(b, ots):
                """Output projection tail, tg-outer so only the final 8
                matmuls wait on the last eg's normalization. PSUM rotates
                through the idle qk+fill rings (4 in flight: no matmul<->cast
                ping-pong, PE stays busy enough to hold full HAM rate); casts
                alternate ScalarE/DVE; writes go in two [128,1024] waves
                (2KB lines) so the drain overlaps the remaining casts."""
                idx = 0
                for tg in range(1, NTG_B):
                    t0 = b * S + tg * TGW
                    for ec in range(NEC):
                        ps2 = psum.tile(
                            [P, TGW], f32,
                            tag=("qk" if idx % 2 == 0 else "fill"),
                            name=f"ps2_{b}_{ec}_{tg}",
                        )
                        idx += 1
                        nc.tensor.matmul(
                            ps2,
                            wo_sb[:, ec * P : (ec + 1) * P],
                            ctxT_sb[:, t0 : t0 + TGW],
                            start=True,
                            stop=True,
                        )
                        dst = ots[ec][:, tg * TGW : (tg + 1) * TGW]
                        if idx % 2 == 0:
                            nc.scalar.activation(dst, ps2[:], COPY)
                        else:
                            nc.vector.tensor_copy(dst, ps2[:])
                    if tg == 1:
                        for ec in range(NEC):
                            nc.sync.dma_start(
                                outT[ec * P : (ec + 1) * P, b * S : b * S + 2 * TGW],
                                ots[ec][:, 0 : 2 * TGW],
                            )
                for ec in range(NEC):
                    nc.sync.dma_start(
                        outT[ec * P : (ec + 1) * P, b * S + 2 * TGW : b * S + 4 * TGW],
                        ots[ec][:, 2 * TGW : 4 * TGW],
                    )

            def vaug_step_filler(b, extra, extra_rate=1):
                """Per-step filler batches for one batch's attention: the
                V-relayout chunks 2..15 (3 ops each, completing 2 steps ahead
                of the AV that reads each chunk), interleaved with `extra`
                ops (one per step), then the remaining extras 2 per step.
                Returns (preblock_ops, step_batches)."""
                vops = list(vaug_ops(b))
                pre, vops = vops[:6], vops[6:]
                extra = list(extra)
                steps = []
                for c in range(14):
                    batch = vops[c * 3 : (c + 1) * 3]
                    for _ in range(min(extra_rate, len(extra))):
                        batch.append(extra.pop(0))
                    steps.append(batch)
                while extra:
                    steps.append([extra.pop(0) for _ in range(min(2, len(extra)))])
                return pre, steps

            # ---- head: k,v,q loads; K/V projections hide in the DMA
            # ---- shadow; Q's first token group gates attention(0).
            xts0 = issue_x_loads(0, ("k", "v", "q"), with_weights=True)
            nc.vector.memset(ones_cols, 1.0)
            # preload the exp activation table set (~2.7us) during the
            # DMA-bound head instead of at the first real exp.
            exp_dummy = singles.tile([P, 1], f16)
            nc.scalar.activation(exp_dummy[:], ones_cols[:, 0, :], EXP)
            for h in range(hpc):
                one_col = h * (dk + 1) + dk
                nc.vector.tensor_copy(vaug_sb[:, :, one_col : one_col + 1], ones_cols[:])

            run_proj_one(0, "k", xts0)
            run_proj_one(0, "v", xts0)
            run_proj_one(0, "q", xts0)
            # attention(0) filler: V-relayout chunks (2 preblocked, rest 3
            # ops/step just ahead of their AVs), Q tg1-3 chains (each done
            # ahead of the eg that reads it), and the x1 DMA batch at eg0's
            # end. proj(1) stays AFTER attention(0) in program order: the
            # static scheduler weaves its chains into remaining slack as x1
            # tiles land.
            pre0, steps0 = vaug_step_filler(0, [], extra_rate=0)
            # x1's tile allocations reuse x0's 16 slots, so each x1 DMA
            # waits until the x0 tiles it replaces have been fully read by
            # every ALREADY-EMITTED reader: x1 never steals HBM bandwidth
            # from x0-q, whose landing gates the first exp. Only x1-k (whose
            # slots' readers, the x0-v chains, are all emitted) may issue
            # before attention(0); x1-v must follow the woven q0 chains and
            # x1-q must follow the k1 projection, else the DMA lands before
            # readers that are emitted later in program order (silent
            # use-after-free -- the dependency tracker only orders against
            # prior program points).
            xts1 = issue_x_loads(1, ("k", "q"))
            xts1.update(issue_x_loads(1, ("v",)))
            for op in pre0:
                op()
            run_attention(0, egs=range(NEG), filler=steps0)
            run_proj_one(1, "k", xts1)
            run_proj_one(1, "q", xts1)
            run_proj_one(1, "v", xts1)
            # k1/v1 projections and q1's first token group are emitted
            # here (the scheduler weaves them into attention(0)'s remaining
            # slack as the x1 tiles land); q1 tg1-3 weave into attention(1)
            # itself, each finishing ahead of the eg that reads it.
            ots1 = []
            ext1 = list(outproj_fill_ops(0))
            ext1.extend(outproj_tail_tg0_ops(1, ots1))
            # attention(1) filler: batch 1's V-relayout first (its AVs need
            # it chunk by chunk), then batch 0's output projection and the
            # tail batch's first token group.
            pre1, steps1 = vaug_step_filler(1, ext1, extra_rate=2)
            for op in pre1:
                op()
            run_attention(1, egs=range(NEG), filler=steps1)
            run_outproj_tail(1, ots1)

    nc.compile()
    return nc


_NC_CACHE = {}


def _compiled():
    if "nc" not in _NC_CACHE:
        _NC_CACHE["nc"] = build_program()
    return _NC_CACHE["nc"]


def _permute_w(w):
    """[D, oc] -> [p, dc, oc] so the device can fetch it with one DMA."""
    D, oc = w.shape
    return np.ascontiguousarray(w.reshape(D // 128, 128, oc).transpose(1, 0, 2))


def make_in_maps(q, k, v, Wq, bq, Wk, bk, Wv, bv, Wo):
    """Shard inputs for the 8 cores: shared transposed activations plus
    per-core head-slice weight columns / Wo rows."""
    f = np.float32
    h16 = np.float16
    qT = np.ascontiguousarray(q.transpose(2, 1, 0).reshape(D_MODEL, -1)).astype(h16)
    kT = np.ascontiguousarray(k.transpose(2, 1, 0).reshape(D_MODEL, -1)).astype(h16)
    vT = np.ascontiguousarray(v.transpose(2, 1, 0).reshape(D_MODEL, -1)).astype(h16)
    Wq, Wk, Wv, Wo = (np.asarray(w).astype(h16) for w in (Wq, Wk, Wv, Wo))
    in_maps = []
    for c in range(N_CORES):
        sl = slice(c * OC, (c + 1) * OC)
        in_maps.append(
            {
                "qT": qT,
                "kT": kT,
                "vT": vT,
                "Wq_s": _permute_w(Wq[:, sl]),
                "Wk_s": _permute_w(Wk[:, sl]),
                "Wv_s": _permute_w(Wv[:, sl]),
                "bq_s": np.ascontiguousarray(bq[sl].reshape(OC, 1), dtype=f),
                "bk_s": np.ascontiguousarray(bk[sl].reshape(OC, 1), dtype=f),
                "bv_s": np.ascontiguousarray(bv[sl].reshape(OC, 1), dtype=f),
                "Wo_s": np.ascontiguousarray(Wo[sl, :]),
            }
        )
    return in_maps


def _install_ntff_shim():
    """Provide antenv.axon_hooks (absent on some images) so that
    trace=True / BASS_TRACE=1 in run_bass_kernel_spmd works instead of
    crashing with ModuleNotFoundError. Best-effort."""
    import sys
    import types

    try:
        import antenv
    except ImportError:
        return
    try:
        import antenv.axon_hooks  # noqa: F401

        return  # real module exists
    except ImportError:
        pass
    mod = types.ModuleType("antenv.axon_hooks")
    mod._hook = None
    mod.set_axon_ntff_profile_hook = lambda h: setattr(mod, "_hook", h)
    mod.get_axon_ntff_profile_hook = lambda: mod._hook
    sys.modules["antenv.axon_hooks"] = mod
    antenv.axon_hooks = mod
    try:
        from trn_agent_boot.trn_boot import _ntff_profile_via_ctypes

        hook = _ntff_profile_via_ctypes("/opt/axon/libaxon_pjrt.so")
        if hook is not None:
            mod.set_axon_ntff_profile_hook(hook)
    except Exception:  # noqa: BLE001
        pass


def kernel(q, k, v, Wq, bq, Wk, bk, Wv, bv, Wo, bo):
    global LAST_RESULTS
    from concourse.bass_utils import run_bass_kernel_spmd

    _install_ntff_shim()

    nc = _compiled()
    in_maps = make_in_maps(q, k, v, Wq, bq, Wk, bk, Wv, bv, Wo)
    res = run_bass_kernel_spmd(nc, in_maps, core_ids=list(range(N_CORES)))
    LAST_RESULTS = res
    total = res.results[0]["outT"].astype(np.float64)
    for c in range(1, N_CORES):
        total += res.results[c]["outT"]
    out = total.reshape(D_MODEL, BATCH, SEQ).transpose(2, 1, 0) + np.asarray(
        bo, dtype=np.float64
    )
    return np.ascontiguousarray(out, dtype=np.float32)
